# revision 1
# baseline (speedup 1.0000x reference)
"""Trainium2 Bass kernel for nn_EMAX_60756607369740.

Computation (per sample b, per group g of 16 channels, over 64x64 maps):
  - coordinate pooling strips -> 1x1 conv (w1) -> sigmoid gates -> x1
  - per-channel GroupNorm stats of x1 (used only through the a21-weighted
    channel contraction; a11 == uniform 1/16 exactly because the GN output
    has zero spatial mean)
  - 3x3 conv branch x2 enters only via (1/16)*sum_c x2 (an 8-output-col
    conv on the 4 PE column tiles, so 4 row-quarters stream concurrently;
    the b21 contraction rides as a 10th tap) and via its pooled
    per-channel sums (reconstructed algebraically from row/col/corner
    sums)
  - wv = (1/16)sum_c x2 + sum_c b21[c] x1[c] - k ; spatial = x*sigmoid(wv)
  - channel SE on x, fuse, global SE on fused.

Sharding: pure data parallel over batch B=16 -> 2 samples per core x 8 cores.
Per-core tile: [128 partitions = 8 groups x 16 ch, 4096 = 64h x 64w].

Perf structure: input staged host-side pre-padded [C, 64, 68] bf16 (one
contiguous DMA descriptor per partition, no on-chip pad handling); output
staged bf16 and upcast on host.  Emission is software-pipelined across the
8 (sample, slab) pairs so each engine's in-order queue always holds ready
work from other slabs.  All sigmoids are tanh-based (sigmoid(x) =
0.5*tanh(x/2)+0.5, scale folded into the broadcast matmul weights and scb)
so the scalar engine stays in one ACT table-set (exp_and_others) with zero
table reloads.  The per-quarter group->channel broadcast runs on the 4 PE
row tiles via maskT4.
"""

import sys

for _p in ("/opt/trn_rl_repo", "/root/.axon_site/_ro/trn_rl_repo"):
    if _p not in sys.path:
        sys.path.insert(0, _p)

import numpy as np
import ml_dtypes

import bass_rust
import concourse.bass as bass
import concourse.mybir as mybir
from concourse.tile import TileContext
from concourse.bass_utils import run_bass_kernel_spmd

F32 = mybir.dt.float32
BF16 = mybir.dt.bfloat16
AF = mybir.ActivationFunctionType
OP = mybir.AluOpType
AX = mybir.AxisListType

B, C, H, W = 16, 512, 64, 64
G, CG, R = 32, 16, 16
EPS = 1e-5
NCORES = 8
BPC = B // NCORES          # samples per core
NSLAB = C // 128           # 4 slabs of 128 channels per sample
HW = H * W                 # 4096
WP = W + 4                 # padded row length 68 (2 left, 2 right zeros)
# taps ordered dy=0 first so the first matmul in each PSUM group covers the
# full chunk (start=True clears the whole region)
TAPS = [(0, -1), (0, 0), (0, 1),
        (-1, -1), (-1, 0), (-1, 1),
        (1, -1), (1, 0), (1, 1)]

MAX_WAITS_PER_INST = 1


def _patched_drain_and_barrier(self, tick_clock, wait_clock):
    # Workaround for walrus "Too many sync wait commands" on the final tile
    # drain: split the aggregated sem waits across many drain instructions.
    drain_inst = self.nc.sync.drain()
    wait_clock.add_sem_waits(
        drain_inst.ins, bass_rust.ScopedClock({None: tick_clock.global_clock})
    )
    mi = drain_inst.ins
    si = mi.sync_info
    if si is not None and len(si.on_wait) > MAX_WAITS_PER_INST:
        waits = list(si.on_wait)
        mi.sync_info = bass_rust.SyncInfo(
            on_wait=waits[:MAX_WAITS_PER_INST], on_update=list(si.on_update)
        )
        rest = waits[MAX_WAITS_PER_INST:]
        for i in range(0, len(rest), MAX_WAITS_PER_INST):
            d2 = self.nc.sync.drain()
            d2.ins.sync_info = bass_rust.SyncInfo(
                on_wait=rest[i : i + MAX_WAITS_PER_INST], on_update=[]
            )
    self.nc.all_engine_barrier()
    popped = self.nc._tile_sem_poison_stack.pop()
    assert popped is self._sem_poison
    self.nc.clear_and_free_semaphores(list(self.sems.allocated().values()))
    self.nc.all_engine_barrier()


TileContext._drain_and_barrier = _patched_drain_and_barrier


def _split_sync_waits(nc, maxw=MAX_WAITS_PER_INST):
    """Walrus rejects instructions carrying more than a couple of sync
    waits. Rebuild each basic block, hoisting excess waits onto freshly
    created same-engine nops placed immediately before the instruction."""
    func = nc.m.functions[0]
    for blk in func.blocks:
        insts = list(blk.instructions)
        need = []
        for inst in insts:
            si = inst.sync_info
            if si is not None and len(si.on_wait) > maxw:
                need.append(inst)
        if not need:
            continue
        donors = {}
        for inst in need:
            si = inst.sync_info
            waits = list(si.on_wait)
            extra = waits[:-maxw] if maxw > 0 else waits
            keep = waits[-maxw:] if maxw > 0 else []
            inst.sync_info = bass_rust.SyncInfo(
                on_wait=keep, on_update=list(si.on_update))
            chunks = [extra[i:i + max(maxw, 1)]
                      for i in range(0, len(extra), max(maxw, 1))]
            nops = []
            for ch in chunks:
                bi = nc.engines[inst.engine].nop()
                ni = bi.ins
                ni.sync_info = bass_rust.SyncInfo(on_wait=ch, on_update=[])
                nops.append(ni)
                # the nop was appended to the current bb; pull it back out
                for fb in func.blocks:
                    fl = list(fb.instructions)
                    if fl and fl[-1] is ni:
                        fb.instructions = fl[:-1]
                        break
            donors[id(inst)] = nops
        out = []
        for inst in insts:
            out.extend(donors.get(id(inst), []))
            out.append(inst)
        blk.instructions = out


def _bf(x):
    return np.ascontiguousarray(x.astype(ml_dtypes.bfloat16))


def _f32(x):
    return np.ascontiguousarray(x.astype(np.float32))


def build_consts(w1, b1, w3, b3, gn_w, gn_b, cg_w1, cg_b1, cg_w2, cg_b2,
                 ga_w1, ga_b1, ga_w2, ga_b2, gamma):
    """Host-side weight transforms. All arrays laid out [partition, free]."""
    c = {}
    # strip 1x1 conv, block-diagonal over 8 groups; /64 folds the W (or H)
    # mean
    Wstrip = np.zeros((128, 128), np.float32)
    for g in range(8):
        # out[(g,o)] = sum_c w1[o,c] * strip[(g,c)] / 64
        Wstrip[g * 16:(g + 1) * 16, g * 16:(g + 1) * 16] = w1.T / 64.0
    c["Wstrip"] = _bf(Wstrip)
    # sigmoid(x) = 0.5*tanh(x/2) + 0.5 everywhere (keeps scalar in one ACT
    # table-set); biases pre-halved for the tanh(scale=0.5) form
    c["b1t2"] = _f32(np.tile(b1, 8)[:, None] / 2.0)

    # big conv tap weights for the pooled-sum reconstruction (1-col rhs)
    W3t = np.zeros((128, 9, 128), np.float32)
    for t, (dy, dx) in enumerate(TAPS):
        blk = w3[:, :, dy + 1, dx + 1].T  # [c_in, c_out]
        for g in range(8):
            W3t[g * 16:(g + 1) * 16, t, g * 16:(g + 1) * 16] = blk
    c["W3sb"] = _bf(W3t.reshape(128, 9 * 128))

    # channel-mean conv weights: out group column g, value sum_c w3 / 16.
    # 8-wide outputs keep the PE in 32-col tiling so 4 quarters run
    # concurrently on the array.
    w3bar = np.zeros((128, 9, 8), np.float32)
    for t, (dy, dx) in enumerate(TAPS):
        vec = w3[:, :, dy + 1, dx + 1].sum(0) / 16.0  # [c_in]
        for g in range(8):
            w3bar[g * 16:(g + 1) * 16, t, g] = vec
    c["w3bar"] = _bf(w3bar.reshape(128, 9 * 8))
    # maskT replicated at the 4 PE row-tile offsets so the 4 quarters'
    # broadcast matmuls run concurrently on (32,128) row tiles.  Entries are
    # 0.5: the broadcast turns tanh into sigmoid (0.5*T; the +0.5 rides in
    # scb)
    maskT4 = np.zeros((128, 128), np.float32)
    for q in range(4):
        for g in range(8):
            maskT4[32 * q + g, g * 16:(g + 1) * 16] = 0.5
    c["maskT4"] = _bf(maskT4)
    mask8 = np.zeros((128, 8), np.float32)
    for g in range(8):
        mask8[g * 16:(g + 1) * 16, g] = 1.0
    c["mask8b"] = _bf(mask8)

    # group block mask: Wg[p, o] = 1 if same group (for per-group sums
    # replicated across the group's 16 channels)
    Wg = np.zeros((128, 128), np.float32)
    for g in range(8):
        Wg[g * 16:(g + 1) * 16, g * 16:(g + 1) * 16] = 1.0
    c["Wg"] = _bf(Wg)
    # per-group sums landing at rows 32q+g for each PE row-tile offset q
    mask32 = np.zeros((128, 128), np.float32)
    for q in range(4):
        for g in range(8):
            mask32[g * 16:(g + 1) * 16, 32 * q + g] = 1.0
    c["mask32"] = _bf(mask32)
    c["magic"] = np.full((128, 1), 0x5f3759df, np.int32)
    c["b3c"] = _f32(np.tile(b3, 8)[:, None])

    # SE dense layers, per 128-channel slab; /4096 folds the HW mean.
    # f32 weights; rhs are 1-column f32 stat vectors.
    cg1 = np.zeros((128, 4, 32), np.float32)
    ga1 = np.zeros((128, 4, 32), np.float32)
    cg2 = np.zeros((32, 4, 128), np.float32)
    ga2 = np.zeros((32, 4, 128), np.float32)
    for s in range(4):
        cg1[:, s, :] = cg_w1[:, s * 128:(s + 1) * 128].T / 4096.0
        ga1[:, s, :] = ga_w1[:, s * 128:(s + 1) * 128].T / 4096.0
        cg2[:, s, :] = cg_w2[s * 128:(s + 1) * 128, :].T
        ga2[:, s, :] = ga_w2[s * 128:(s + 1) * 128, :].T
    c["cg1w"] = _bf(cg1.reshape(128, 128))
    c["ga1w"] = _bf(ga1.reshape(128, 128))
    c["cg2w"] = _bf(cg2.reshape(32, 512))
    c["ga2w"] = _bf(ga2.reshape(32, 512))
    c["cgb1"] = _f32(cg_b1[:, None])
    c["gab1"] = _f32(ga_b1[:, None])
    c["cgb2h"] = _f32(cg_b2.reshape(4, 128).T / 2.0)
    c["gab2h"] = _f32(ga_b2.reshape(4, 128).T / 2.0)

    c["gnw"] = _f32(np.tile(gn_w, 8)[:, None])
    c["gnb"] = _f32(np.tile(gn_b, 8)[:, None])
    c["epsc"] = _f32(np.full((128, 1), EPS, np.float32))
    return c


def build_program(gamma_f, mean_b3_f, const_shapes):
    nc = bass.Bass("TRN2", target_bir_lowering=False, debug=False,
                   num_devices=NCORES)
    xp_d = nc.dram_tensor("xp", [BPC, C, H, WP], BF16, kind="ExternalInput")
    y_d = nc.dram_tensor("y", [BPC, C, H, W], BF16, kind="ExternalOutput")
    cd = {}
    for name, (shape, dt) in const_shapes.items():
        cd[name] = nc.dram_tensor(name, list(shape), dt, kind="ExternalInput")

    with TileContext(nc) as tc:
        with (
            tc.sbuf_pool(name="consts", bufs=1) as cpool,
            tc.sbuf_pool(name="big", bufs=2) as bpool,
            tc.sbuf_pool(name="fusedp", bufs=NSLAB + 1) as fpool,
            tc.sbuf_pool(name="small", bufs=2) as spool,
            tc.psum_pool(name="pwv", bufs=2) as pwv,
            tc.psum_pool(name="prep", bufs=2) as prep,
            tc.psum_pool(name="psmall", bufs=2) as psmall,
        ):
            cs = {}
            for name in const_shapes:
                t = cpool.tile(list(cd[name].shape), cd[name].dtype, name=f"c_{name}")
                nc.sync.dma_start(t[:, :], cd[name].ap())
                cs[name] = t

            emit_all(nc, tc, xp_d, y_d, cs, bpool, fpool, spool,
                     pwv, prep, psmall, gamma_f, mean_b3_f)
    _split_sync_waits(nc)
    return nc


def emit_all(nc, tc, xp_d, y_d, cs, bpool, fpool, spool,
             pwv, prep, psmall, gamma_f, mean_b3_f):
    """Software-pipelined emission: per-engine queues execute in program
    order, so stages of different (sample, slab) pairs are interleaved to
    fill each engine's stalls with ready work from other slabs."""
    sc = nc.scalar
    ve = nc.vector
    gp = nc.gpsimd
    te = nc.tensor

    def custom_ap(base_ap, extra_off, free_dims):
        p0 = list(base_ap.ap[0])
        return bass.AP(base_ap.tensor, base_ap.offset + extra_off,
                       [p0] + [list(d) for d in free_dims])

    # per-sample small stats tiles ([*, slab] columns)
    S = []
    for b in range(BPC):
        st = dict(
            xsum=spool.tile([128, 4], F32, name=f"xsum{b}"),
            x1sum=spool.tile([128, 4], F32, name=f"x1sum{b}"),
            x1sq=spool.tile([128, 4], F32, name=f"x1sq{b}"),
            fsum=spool.tile([128, 4], F32, name=f"fsum{b}"),
            edges=spool.tile([128, 16], F32, name=f"edges{b}"),
            corners=spool.tile([128, 16], BF16, name=f"corners{b}"),
            mu=spool.tile([128, 4], F32, name=f"mu{b}"),
            rstdw=spool.tile([128, 4], F32, name=f"rstdw{b}"),
            b21=spool.tile([128, 4], F32, name=f"b21{b}"),
            biaswv=spool.tile([128, 4], F32, name=f"biaswv{b}"),
            scb=spool.tile([128, 4], F32, name=f"scb{b}"),
            gga=spool.tile([128, 4], F32, name=f"gga{b}"),
            xpad={}, strips={}, x1={}, fused={}, b21blk={},
        )
        S.append(st)

    def stage_a(b, s):
        """Load slab, strip sums, pooled sum, edge/corner gathers."""
        st = S[b]
        xpad = bpool.tile([128, H * WP], BF16, name="xpad", tag="xpad",
                          bufs=5)
        st["xpad"][s] = xpad
        strips = spool.tile([128, 128], F32, name="strips", tag="strips",
                            bufs=5)
        st["strips"][s] = strips
        srcp = xp_d.ap()[b, s * 128:(s + 1) * 128, :, :]
        srcp = bass.AP(srcp.tensor, srcp.offset,
                       [list(srcp.ap[0]), [1, H * WP]])
        gp.dma_start(xpad[:, :], srcp)

        xp3 = xpad[:, :].rearrange("p (h w) -> p h w", w=WP)
        tree = spool.tile([128, 3072], BF16, name="tree", tag="tree",
                          bufs=2)
        # row sums over w -> strips[:, 0:64] (the "xh" strip)
        t3 = tree[:, 0:2048].rearrange("p (h w) -> p h w", w=32)
        t4 = tree[:, 2048:3072].rearrange("p (h w) -> p h w", w=16)
        ve.tensor_tensor(t3, xp3[:, :, 2:34], xp3[:, :, 34:66], OP.add)
        ve.tensor_tensor(t4, t3[:, :, 0:16], t3[:, :, 16:32], OP.add)
        ve.tensor_reduce(strips[:, 0:64], t4, AX.X, OP.add)
        # col sums over h -> strips[:, 64:128] (the "xw" strip)
        t3w = tree[:, 0:2048].rearrange("p (h w) -> p h w", w=W)[:, 0:32, :]
        t4w = tree[:, 2048:3072].rearrange("p (h w) -> p h w", w=W)[:, 0:16, :]
        ve.tensor_tensor(t3w, xp3[:, 0:32, 2:66], xp3[:, 32:64, 2:66],
                         OP.add)
        ve.tensor_tensor(t4w, t3w[:, 0:16, :], t3w[:, 16:32, :], OP.add)
        ve.tensor_reduce(strips[:, 64:128], t4w.transpose([0, 2, 1]), AX.X,
                         OP.add)
        ve.tensor_reduce(st["xsum"][:, s:s + 1], strips[:, 0:64], AX.X,
                         OP.add)
        # edge strip values {h=0, h=63, w=0, w=63}
        eap = custom_ap(strips[:, :], 0, [[64, 2], [63, 2]])
        ve.tensor_copy(
            st["edges"][:, 4 * s:4 * s + 4].rearrange("p (a c) -> p a c",
                                                      c=2), eap)
        # corner values {(0,0), (0,63), (63,0), (63,63)}
        cap = custom_ap(xpad[:, :], 2, [[63 * WP, 2], [63, 2]])
        ve.tensor_copy(
            st["corners"][:, 4 * s:4 * s + 4].rearrange("p (a c) -> p a c",
                                                        c=2), cap)

    def stage_se(b):
        """Channel SE gate (needs all 4 slab xsums)."""
        st = S[b]
        xsumbf = spool.tile([128, 4], BF16, name="xsumbf", tag="xsumbf")
        sc.activation(xsumbf[:, :], st["xsum"][:, :], AF.Copy)
        h1 = psmall.tile([32, 1], F32, name="h1", tag="ps_a")
        for s in range(NSLAB):
            te.matmul(h1[:, :], cs["cg1w"][:, 32 * s:32 * s + 32],
                      xsumbf[:, s:s + 1], start=(s == 0), stop=(s == 3))
        hid = spool.tile([32, 1], BF16, name="hid", tag="hid")
        sc.activation(hid[:, :], h1[:, :], AF.Relu, bias=cs["cgb1"][:, 0:1])
        gps = psmall.tile([128, 4], F32, name="gps", tag="ps_b")
        for s in range(NSLAB):
            te.matmul(gps[:, s:s + 1], cs["cg2w"][:, 128 * s:128 * s + 128],
                      hid[:, :], start=True, stop=True)
        for s in range(NSLAB):
            sc.activation(st["scb"][:, s:s + 1], gps[:, s:s + 1], AF.Tanh,
                          scale=0.5, bias=cs["cgb2h"][:, s:s + 1])
        # scb = gamma*sigmoid + 0.5 (the +0.5 completes the wv tanh->sigmoid
        # conversion inside the fused op; maskT4 carries the 0.5 factor)
        ve.tensor_scalar(st["scb"][:, :], st["scb"][:, :],
                         float(gamma_f) / 2.0, float(gamma_f) / 2.0 + 0.5,
                         OP.mult, OP.add)

    def stage_b(b, s):
        """Strip conv -> gates -> x1 map + its moment accumulators."""
        st = S[b]
        xpad = st["xpad"][s]
        strips = st["strips"][s]
        xp3 = xpad[:, :].rearrange("p (h w) -> p h w", w=WP)
        xpi3 = xp3[:, :, 2:W + 2]

        stripbf = spool.tile([128, 128], BF16, name="stripbf", tag="stripbf")
        sc.activation(stripbf[:, :], strips[:, :], AF.Copy)
        strip_ps = psmall.tile([128, 128], F32, name="strip_ps", tag="ps_a")
        te.matmul(strip_ps[:, :], cs["Wstrip"][:, :], stripbf[:, :],
                  start=True, stop=True)
        sgate = spool.tile([128, 128], BF16, name="sgate", tag="sgate",
                           bufs=3)
        sc.activation(sgate[:, :], strip_ps[:, :], AF.Tanh, scale=0.5,
                      bias=cs["b1t2"][:, 0:1])
        ve.tensor_scalar(sgate[:, :], sgate[:, :], 0.5, 0.5, OP.mult,
                         OP.add)

        x1 = bpool.tile([128, HW], BF16, name="x1", tag="x1", bufs=3)
        st["x1"][s] = x1
        x13 = x1[:, :].rearrange("p (h w) -> p h w", w=W)
        wg = sgate[:, 64:128].unsqueeze(1).broadcast_to((128, H, W))
        ve.tensor_tensor(x13, xpi3, wg, OP.mult)
        hg = sgate[:, 0:64].unsqueeze(2).broadcast_to((128, H, W))
        ve.scalar_tensor_tensor(x13, x13, 0.0, hg, OP.add, OP.mult,
                                accum_out=st["x1sum"][:, s:s + 1])
        junk = bpool.tile([128, HW], BF16, name="junk", tag="scratch",
                          bufs=2)
        sc.activation(junk[:, :], x1[:, :], AF.Square,
                      accum_out=st["x1sq"][:, s:s + 1])

    def stage_c(b, s):
        """GroupNorm stats, pooled-softmax coefficients, wv bias."""
        st = S[b]
        mu, rstdw, b21 = st["mu"], st["rstdw"], st["b21"]
        gp.tensor_scalar_mul(mu[:, s:s + 1], st["x1sum"][:, s:s + 1],
                             1.0 / HW)
        var = spool.tile([128, 1], F32, name="var", tag="var")
        gp.tensor_tensor(var[:, :], mu[:, s:s + 1], mu[:, s:s + 1], OP.mult)
        ve.scalar_tensor_tensor(var[:, :], st["x1sq"][:, s:s + 1], 1.0 / HW,
                                var[:, :], OP.mult, OP.subtract)
        ve.tensor_scalar_add(var[:, :], var[:, :], float(EPS))
        # rstd = rsqrt(var) via bit trick + 1 Newton iteration (~1.7e-3 rel)
        ti = spool.tile([128, 1], mybir.dt.int32, name="ti", tag="ti")
        ve.tensor_scalar(ti[:, :], var[:, :].bitcast(mybir.dt.int32), 1,
                         None, OP.logical_shift_right)
        ve.tensor_tensor(ti[:, :], cs["magic"][:, 0:1], ti[:, :],
                         OP.subtract)
        ry = spool.tile([128, 1], F32, name="ry", tag="ry")
        rt = spool.tile([128, 1], F32, name="rt", tag="rt")
        ve.tensor_copy(ry[:, :], ti[:, :].bitcast(F32))
        gp.tensor_tensor(rt[:, :], ry[:, :], ry[:, :], OP.mult)
        gp.tensor_tensor(rt[:, :], rt[:, :], var[:, :], OP.mult)
        gp.tensor_scalar(rt[:, :], rt[:, :], -0.5, 1.5, OP.mult, OP.add)
        gp.tensor_tensor(ry[:, :], ry[:, :], rt[:, :], OP.mult)
        gp.tensor_tensor(rstdw[:, s:s + 1], ry[:, :], cs["gnw"][:, 0:1],
                         OP.mult)

        # pooled x2 sums via edge algebra: A[p, tap]
        A = spool.tile([128, 9], F32, name="A", tag="A")
        Ap = A[:, :]
        gp.tensor_copy(Ap, st["xsum"][:, s:s + 1].broadcast_to((128, 9)))
        out_r = custom_ap(Ap, 3, [[3, 2], [1, 3]])
        in_r = custom_ap(st["edges"][:, :], 4 * s + 1, [[-1, 2], [0, 3]])
        gp.tensor_tensor(out_r, out_r, in_r, OP.subtract)
        out_c1 = custom_ap(Ap, 0, [[3, 3]])
        in_c1 = custom_ap(st["edges"][:, :], 4 * s + 3, [[0, 3]])
        gp.tensor_tensor(out_c1, out_c1, in_c1, OP.subtract)
        out_c2 = custom_ap(Ap, 2, [[3, 3]])
        in_c2 = custom_ap(st["edges"][:, :], 4 * s + 2, [[0, 3]])
        gp.tensor_tensor(out_c2, out_c2, in_c2, OP.subtract)
        out_k1 = custom_ap(Ap, 3, [[2, 2]])
        in_k1 = custom_ap(st["corners"][:, :], 4 * s + 3, [[-1, 2]])
        gp.tensor_tensor(out_k1, out_k1, in_k1, OP.add)
        out_k2 = custom_ap(Ap, 6, [[2, 2]])
        in_k2 = custom_ap(st["corners"][:, :], 4 * s + 1, [[-1, 2]])
        gp.tensor_tensor(out_k2, out_k2, in_k2, OP.add)
        Abf = spool.tile([128, 9], BF16, name="Abf", tag="Abf")
        sc.activation(Abf[:, :], A[:, :], AF.Copy)
        p2 = psmall.tile([128, 1], F32, name="p2", tag="ps_a")
        for t in range(9):
            te.matmul(p2[:, :], cs["W3sb"][:, 128 * t:128 * t + 128],
                      Abf[:, t:t + 1], start=(t == 0), stop=(t == 8))
        # e4 = exp(p2/HW + b3) via the ACT LUT; the Copy-to-bf16 rides along
        e4 = spool.tile([128, 1], F32, name="e4", tag="e4")
        sc.activation(e4[:, :], p2[:, :], AF.Exp, scale=1.0 / HW,
                      bias=cs["b3c"][:, 0:1])
        ebf = spool.tile([128, 1], BF16, name="ebf", tag="ebf")
        sc.activation(ebf[:, :], e4[:, :], AF.Copy)
        # softmax denominator per group, replicated to all 128 rows
        rs8 = psmall.tile([128, 1], F32, name="rs8", tag="ps_b")
        te.matmul(rs8[:, :], cs["Wg"][:, :], ebf[:, :], start=True,
                  stop=True)
        rec = spool.tile([128, 1], F32, name="rec", tag="rec")
        ve.reciprocal(rec[:, :], rs8[:, :])
        a21 = spool.tile([128, 1], F32, name="a21", tag="a21")
        gp.tensor_tensor(a21[:, :], e4[:, :], rec[:, :], OP.mult)
        gp.tensor_tensor(b21[:, s:s + 1], a21[:, :], rstdw[:, s:s + 1],
                         OP.mult)
        kv = spool.tile([128, 1], F32, name="kv", tag="kv")
        gp.tensor_tensor(kv[:, :], b21[:, s:s + 1], mu[:, s:s + 1], OP.mult)
        k2 = spool.tile([128, 1], BF16, name="k2", tag="k2")
        ve.scalar_tensor_tensor(k2[:, :], a21[:, :], cs["gnb"][:, 0:1],
                                kv[:, :], OP.mult, OP.subtract)
        kps = psmall.tile([128, 1], F32, name="kps", tag="ps_a")
        te.matmul(kps[:, :], cs["mask32"][:, :], k2[:, :], start=True,
                  stop=True)
        # pre-halved for the tanh(scale=0.5) sigmoid form
        sc.activation(st["biaswv"][:, s:s + 1], kps[:, :], AF.Copy,
                      scale=0.5, bias=float(mean_b3_f) / 2.0)
        b21blk = spool.tile([128, 8], BF16, name="b21blk", tag="b21blk",
                            bufs=3)
        st["b21blk"][s] = b21blk
        sc.activation(b21blk[:, :], cs["mask8b"][:, :], AF.Copy,
                      scale=b21[:, s:s + 1])

    def stage_d(b, s):
        """wv conv + contraction, sigmoid, broadcast, fused map."""
        st = S[b]
        xpad = st["xpad"][s]
        x1 = st["x1"][s]
        b21blk = st["b21blk"][s]
        biaswv = st["biaswv"]
        scb = st["scb"]
        xp3 = xpad[:, :].rearrange("p (h w) -> p h w", w=WP)
        xpi3 = xp3[:, :, 2:W + 2]

        fused = fpool.tile([128, HW], BF16, name="fused", tag="fused")
        st["fused"][s] = fused
        fparts = spool.tile([128, 8], F32, name="fparts", tag="fparts")
        wvs = []
        sigs = []
        # both halves' conv groups issue back-to-back so the PE stream stays
        # warm; the b21 contraction rides as a 10th tap inside the loop
        for half in range(2):
            wv = pwv.tile([128, 512], F32, name="wv", tag="wv")
            wvs.append(wv)
            for t in range(10):
                for q in range(4):
                    hc = 4 * half + q
                    h0 = hc * 8
                    if t == 9:
                        te.matmul(wv[32 * q:32 * q + 8, :], b21blk[:, :],
                                  x1[:, 512 * hc:512 * hc + 512],
                                  start=False, stop=True,
                                  tile_position=(0, 32 * q))
                        continue
                    dy, dx = TAPS[t]
                    i0 = max(0, -(h0 + dy))
                    i1 = min(8, 64 - h0 - dy)
                    rhs = xp3[:, h0 + i0 + dy:h0 + i1 + dy,
                              2 + dx:2 + dx + W]
                    te.matmul(wv[32 * q:32 * q + 8, i0 * 64:i1 * 64],
                              cs["w3bar"][:, 8 * t:8 * t + 8], rhs,
                              start=(t == 0), stop=False,
                              tile_position=(0, 32 * q))
            # one sigmoid covers all 4 row-tiles; per-block bias rides the
            # [128,1] bias AP
            sig = spool.tile([128, 512], BF16, name="sig", tag="sig",
                             bufs=4)
            sigs.append(sig)
            sc.activation(sig[:, :], wv[:, :], AF.Tanh, scale=0.5,
                          bias=biaswv[:, s:s + 1])
        for half in range(2):
            sig = sigs[half]
            for q in range(4):
                hc = 4 * half + q
                repq = prep.tile([128, 512], F32, name="repq", tag="repq")
                te.matmul(repq[:, :], cs["maskT4"][32 * q:32 * q + 8, :],
                          sig[32 * q:32 * q + 8, :],
                          start=True, stop=True,
                          tile_position=(32 * q, 0))
                ve.scalar_tensor_tensor(
                    fused[:, 512 * hc:512 * hc + 512], repq[:, :],
                    scb[:, s:s + 1],
                    xpi3[:, 8 * hc:8 * hc + 8, :],
                    OP.add, OP.mult, accum_out=fparts[:, hc:hc + 1])
        ve.tensor_reduce(st["fsum"][:, s:s + 1], fparts[:, :], AX.X, OP.add)

    def stage_ga(b):
        """Global-attn SE over fused."""
        st = S[b]
        fsumbf = spool.tile([128, 4], BF16, name="fsumbf", tag="fsumbf")
        sc.activation(fsumbf[:, :], st["fsum"][:, :], AF.Copy)
        h2 = psmall.tile([32, 1], F32, name="h2", tag="ps_a")
        for s in range(NSLAB):
            te.matmul(h2[:, :], cs["ga1w"][:, 32 * s:32 * s + 32],
                      fsumbf[:, s:s + 1], start=(s == 0), stop=(s == 3))
        hid2 = spool.tile([32, 1], BF16, name="hid2", tag="hid")
        sc.activation(hid2[:, :], h2[:, :], AF.Relu, bias=cs["gab1"][:, 0:1])
        gps2 = psmall.tile([128, 4], F32, name="gps2", tag="ps_b")
        for s in range(NSLAB):
            te.matmul(gps2[:, s:s + 1], cs["ga2w"][:, 128 * s:128 * s + 128],
                      hid2[:, :], start=True, stop=True)
        for s in range(NSLAB):
            sc.activation(st["gga"][:, s:s + 1], gps2[:, s:s + 1],
                          AF.Tanh, scale=0.5, bias=cs["gab2h"][:, s:s + 1])
        ve.tensor_scalar(st["gga"][:, :], st["gga"][:, :], 0.5, 0.5,
                         OP.mult, OP.add)

    def stage_e(b, s):
        """Final gate + store (split scalar/vector)."""
        st = S[b]
        fused = st["fused"][s]
        if s < 2:
            sc.activation(fused[:, :], fused[:, :], AF.Copy,
                          scale=st["gga"][:, s:s + 1])
        else:
            ve.tensor_scalar(fused[:, :], fused[:, :],
                             st["gga"][:, s:s + 1], None, OP.mult)
        dst = y_d.ap()[b, s * 128:(s + 1) * 128, :, :]
        dst = bass.AP(dst.tensor, dst.offset, [list(dst.ap[0]), [1, HW]])
        nc.sync.dma_start(dst, fused[:, :])

    A, B, Cs, D, E = stage_a, stage_b, stage_c, stage_d, stage_e
    SE, GA = stage_se, stage_ga
    sched = [
        (A, 0, 0), (A, 0, 1), (B, 0, 0), (A, 0, 2), (B, 0, 1), (Cs, 0, 0),
        (A, 0, 3), (SE, 0), (B, 0, 2), (Cs, 0, 1), (D, 0, 0),
        (B, 0, 3), (Cs, 0, 2), (D, 0, 1), (A, 1, 0), (Cs, 0, 3), (D, 0, 2),
        (A, 1, 1), (B, 1, 0), (D, 0, 3), (GA, 0),
        (A, 1, 2), (B, 1, 1), (Cs, 1, 0), (E, 0, 0), (A, 1, 3), (SE, 1),
        (B, 1, 2), (Cs, 1, 1), (D, 1, 0), (E, 0, 1),
        (B, 1, 3), (Cs, 1, 2), (D, 1, 1), (E, 0, 2), (Cs, 1, 3), (D, 1, 2),
        (E, 0, 3), (D, 1, 3), (GA, 1),
        (E, 1, 0), (E, 1, 1), (E, 1, 2), (E, 1, 3),
    ]
    for entry in sched:
        fn = entry[0]
        fn(*entry[1:])


def _ensure_ntff_hook():
    """run_bass_kernel_spmd(trace=True) under axon needs
    antenv.axon_hooks, which this image's antenv lacks. Shim it and
    register the ctypes-based NTFF hook from the boot package."""
    import types
    try:
        from antenv import axon_hooks  # noqa: F401
        return
    except ImportError:
        pass
    try:
        import antenv
        from trn_agent_boot.trn_boot import _ntff_profile_via_ctypes
        hooks = types.ModuleType("antenv.axon_hooks")
        _h = [None]
        hooks.set_axon_ntff_profile_hook = lambda h: _h.__setitem__(0, h)
        hooks.get_axon_ntff_profile_hook = lambda: _h[0]
        sys.modules["antenv.axon_hooks"] = hooks
        antenv.axon_hooks = hooks
        hooks.set_axon_ntff_profile_hook(
            _ntff_profile_via_ctypes("/opt/axon/libaxon_pjrt.so"))
    except Exception as e:  # profiling is best-effort
        print(f"ntff hook setup failed: {e}")


_CACHE = {}


def _get_program(consts, gamma_f, mean_b3_f):
    key = (float(gamma_f), float(mean_b3_f),
           tuple(sorted((k, v.tobytes()[:64].hex() if v.size > 16 else
                         v.tobytes().hex()) for k, v in consts.items())))
    key = hash(key)
    if key not in _CACHE:
        def _dt(v):
            if v.dtype == ml_dtypes.bfloat16:
                return BF16
            if v.dtype == np.int32:
                return mybir.dt.int32
            return F32
        const_shapes = {k: (v.shape, _dt(v)) for k, v in consts.items()}
        _CACHE[key] = build_program(gamma_f, mean_b3_f, const_shapes)
    return _CACHE[key]


def kernel(x, w1, b1, w3, b3, gn_w, gn_b, cg_w1, cg_b1, cg_w2, cg_b2,
           ga_w1, ga_b1, ga_w2, ga_b2, gamma, _return_timing=None):
    args = [np.asarray(a) for a in
            (x, w1, b1, w3, b3, gn_w, gn_b, cg_w1, cg_b1, cg_w2, cg_b2,
             ga_w1, ga_b1, ga_w2, ga_b2, gamma)]
    (x, w1, b1, w3, b3, gn_w, gn_b, cg_w1, cg_b1, cg_w2, cg_b2,
     ga_w1, ga_b1, ga_w2, ga_b2, gamma) = args
    consts = build_consts(w1, b1, w3, b3, gn_w, gn_b, cg_w1, cg_b1, cg_w2,
                          cg_b2, ga_w1, ga_b1, ga_w2, ga_b2, gamma)
    gamma_f = float(np.asarray(gamma).reshape(-1)[0])
    mean_b3_f = float(np.mean(b3))
    nc = _get_program(consts, gamma_f, mean_b3_f)

    xbf = x.astype(ml_dtypes.bfloat16)           # [B, C, H, W]
    xpad = np.zeros((B, C, H, WP), ml_dtypes.bfloat16)
    xpad[:, :, :, 2:2 + W] = xbf

    in_maps = []
    for core in range(NCORES):
        sl = slice(core * BPC, (core + 1) * BPC)
        m = {"xp": np.ascontiguousarray(xpad[sl])}
        m.update(consts)
        in_maps.append(m)
    trace = bool(_return_timing is not None)
    if trace:
        _ensure_ntff_hook()
    last_err = None
    for _attempt in range(3):
        try:
            res = run_bass_kernel_spmd(nc, in_maps,
                                       core_ids=list(range(NCORES)),
                                       trace=trace)
            break
        except Exception as e:  # transient NRT device errors: retry
            last_err = e
    else:
        raise last_err
    if _return_timing is not None:
        _return_timing.update(dict(
            exec_time_ns=res.exec_time_ns,
            mean_exec_time_ns=res.mean_exec_time_ns,
        ))
    out = np.empty((B, C, H, W), np.float32)
    for core in range(NCORES):
        out[core * BPC:(core + 1) * BPC] = res.results[core]["y"].astype(
            np.float32)
    return out



# revision 7
# speedup vs baseline: 1.0494x; 1.0494x over previous
"""Trainium2 Bass kernel for nn_EMAX_60756607369740.

Computation (per sample b, per group g of 16 channels, over 64x64 maps):
  - coordinate pooling strips -> 1x1 conv (w1) -> sigmoid gates -> x1
  - per-channel GroupNorm stats of x1 (used only through the a21-weighted
    channel contraction; a11 == uniform 1/16 exactly because the GN output
    has zero spatial mean)
  - 3x3 conv branch x2 enters only via (1/16)*sum_c x2 and via its pooled
    per-channel sums (reconstructed algebraically from row/col/corner sums)
  - wv = (1/16)sum_c x2 + sum_c b21[c] x1[c] - k ; spatial = x*sigmoid(wv)
  - channel SE on x, fuse, global SE on fused.

Sharding: pure data parallel over batch B=16 -> 2 samples per core x 8 cores.
Per-core tile: [128 partitions = 8 groups x 16 ch, 4096 = 64h x 64w].

v2 layout/engine choices (vs the 233us baseline):
  - input staged host-side pre-padded [C, 66, 68] bf16 (1 h-pad row and
    2 w-pad cols per side) so every conv-tap matmul is a uniform in-bounds
    512-col rhs slice -- no edge clamping, no 448-col stragglers.
  - strip sums via deep bf16 fold trees at DVE 2x rate (no 1x
    tensor_reduce tails); strips kept bf16 so the strip-conv rhs needs no
    f32->bf16 staging copy.
  - repq (group->channel sigmoid broadcast) matmuls land in one 4-bank
    [128,2048] PSUM tile, so the fused-map scalar_tensor_tensor runs as 2
    ops/slab (PSUM 120-cyc startup paid 2x not 8x).
  - final gga gate on vector tensor_scalar (bf16 4x mode) for all slabs;
    scalar engine keeps Square/sigmoids only.
  - tiny stage-c chain ops placed per measured per-op cost (a21 multiply
    on DVE, not gpsimd).
  - all sigmoids tanh-based so the scalar engine stays in one ACT
    table-set (exp_and_others).
"""

import sys

for _p in ("/opt/trn_rl_repo", "/root/.axon_site/_ro/trn_rl_repo"):
    if _p not in sys.path:
        sys.path.insert(0, _p)

import numpy as np
import ml_dtypes

import bass_rust
import concourse.bass as bass
import concourse.mybir as mybir
from concourse.tile import TileContext
from concourse.bass_utils import run_bass_kernel_spmd

F32 = mybir.dt.float32
BF16 = mybir.dt.bfloat16
AF = mybir.ActivationFunctionType
OP = mybir.AluOpType
AX = mybir.AxisListType

B, C, H, W = 16, 512, 64, 64
G, CG, R = 32, 16, 16
EPS = 1e-5
NCORES = 8
BPC = B // NCORES          # samples per core
NSLAB = C // 128           # 4 slabs of 128 channels per sample
HW = H * W                 # 4096
WP = W + 4                 # padded row length 68 (2 left, 2 right zeros)
HP = H + 2                 # padded row count 66 (1 top, 1 bottom zero row)
# taps ordered dy=0 first so the first matmul in each PSUM group covers the
# full chunk (start=True clears the whole region)
TAPS = [(0, -1), (0, 0), (0, 1),
        (-1, -1), (-1, 0), (-1, 1),
        (1, -1), (1, 0), (1, 1)]

MAX_WAITS_PER_INST = 1


def _patched_drain_and_barrier(self, tick_clock, wait_clock):
    # Workaround for walrus "Too many sync wait commands" on the final tile
    # drain: split the aggregated sem waits across many drain instructions.
    drain_inst = self.nc.sync.drain()
    wait_clock.add_sem_waits(
        drain_inst.ins, bass_rust.ScopedClock({None: tick_clock.global_clock})
    )
    mi = drain_inst.ins
    si = mi.sync_info
    if si is not None and len(si.on_wait) > MAX_WAITS_PER_INST:
        waits = list(si.on_wait)
        mi.sync_info = bass_rust.SyncInfo(
            on_wait=waits[:MAX_WAITS_PER_INST], on_update=list(si.on_update)
        )
        rest = waits[MAX_WAITS_PER_INST:]
        for i in range(0, len(rest), MAX_WAITS_PER_INST):
            d2 = self.nc.sync.drain()
            d2.ins.sync_info = bass_rust.SyncInfo(
                on_wait=rest[i : i + MAX_WAITS_PER_INST], on_update=[]
            )
    self.nc.all_engine_barrier()
    popped = self.nc._tile_sem_poison_stack.pop()
    assert popped is self._sem_poison
    self.nc.clear_and_free_semaphores(list(self.sems.allocated().values()))
    self.nc.all_engine_barrier()


TileContext._drain_and_barrier = _patched_drain_and_barrier


def _split_sync_waits(nc, maxw=MAX_WAITS_PER_INST):
    """Walrus rejects instructions carrying more than a couple of sync
    waits. Rebuild each basic block, hoisting excess waits onto freshly
    created same-engine nops placed immediately before the instruction."""
    func = nc.m.functions[0]
    for blk in func.blocks:
        insts = list(blk.instructions)
        need = []
        for inst in insts:
            si = inst.sync_info
            if si is not None and len(si.on_wait) > maxw:
                need.append(inst)
        if not need:
            continue
        donors = {}
        for inst in need:
            si = inst.sync_info
            waits = list(si.on_wait)
            extra = waits[:-maxw] if maxw > 0 else waits
            keep = waits[-maxw:] if maxw > 0 else []
            inst.sync_info = bass_rust.SyncInfo(
                on_wait=keep, on_update=list(si.on_update))
            chunks = [extra[i:i + max(maxw, 1)]
                      for i in range(0, len(extra), max(maxw, 1))]
            nops = []
            for ch in chunks:
                bi = nc.engines[inst.engine].nop()
                ni = bi.ins
                ni.sync_info = bass_rust.SyncInfo(on_wait=ch, on_update=[])
                nops.append(ni)
                # the nop was appended to the current bb; pull it back out
                for fb in func.blocks:
                    fl = list(fb.instructions)
                    if fl and fl[-1] is ni:
                        fb.instructions = fl[:-1]
                        break
            donors[id(inst)] = nops
        out = []
        for inst in insts:
            out.extend(donors.get(id(inst), []))
            out.append(inst)
        blk.instructions = out


def _bf(x):
    return np.ascontiguousarray(x.astype(ml_dtypes.bfloat16))


def _f32(x):
    return np.ascontiguousarray(x.astype(np.float32))


def build_consts(w1, b1, w3, b3, gn_w, gn_b, cg_w1, cg_b1, cg_w2, cg_b2,
                 ga_w1, ga_b1, ga_w2, ga_b2, gamma):
    """Host-side weight transforms. All arrays laid out [partition, free]."""
    c = {}
    # strip 1x1 conv, block-diagonal over 8 groups; /64 folds the W (or H)
    # mean
    Wstrip = np.zeros((128, 128), np.float32)
    for g in range(8):
        # out[(g,o)] = sum_c w1[o,c] * strip[(g,c)] / 64
        Wstrip[g * 16:(g + 1) * 16, g * 16:(g + 1) * 16] = w1.T / 64.0
    c["Wstrip"] = _bf(Wstrip)
    # sigmoid(x) = 0.5*tanh(x/2) + 0.5 everywhere (keeps scalar in one ACT
    # table-set); biases pre-halved for the tanh(scale=0.5) form
    c["b1t2"] = _f32(np.tile(b1, 8)[:, None] / 2.0)

    # big conv tap weights for the pooled-sum reconstruction (1-col rhs)
    W3t = np.zeros((128, 9, 128), np.float32)
    for t, (dy, dx) in enumerate(TAPS):
        blk = w3[:, :, dy + 1, dx + 1].T  # [c_in, c_out]
        for g in range(8):
            W3t[g * 16:(g + 1) * 16, t, g * 16:(g + 1) * 16] = blk
    c["W3sb"] = _bf(W3t.reshape(128, 9 * 128))

    # channel-mean conv weights: out group column g, value sum_c w3 / 16.
    # 8-wide outputs keep each matmul inside one 32-col PE tile.
    w3bar = np.zeros((128, 9, 8), np.float32)
    for t, (dy, dx) in enumerate(TAPS):
        vec = w3[:, :, dy + 1, dx + 1].sum(0) / 16.0  # [c_in]
        for g in range(8):
            w3bar[g * 16:(g + 1) * 16, t, g] = vec
    c["w3bar"] = _bf(w3bar.reshape(128, 9 * 8))
    # maskT replicated at the 4 PE row-tile offsets so the 4 quarters'
    # broadcast matmuls run on (32,128) row tiles.  Entries are 0.5: the
    # broadcast turns tanh into sigmoid (0.5*T; the +0.5 rides in scb)
    maskT4 = np.zeros((128, 128), np.float32)
    for q in range(4):
        for g in range(8):
            maskT4[32 * q + g, g * 16:(g + 1) * 16] = 0.5
    c["maskT4"] = _bf(maskT4)
    mask8 = np.zeros((128, 8), np.float32)
    for g in range(8):
        mask8[g * 16:(g + 1) * 16, g] = 1.0
    c["mask8b"] = _bf(mask8)

    # group block mask: Wg[p, o] = 1 if same group (for per-group sums
    # replicated across the group's 16 channels)
    Wg = np.zeros((128, 128), np.float32)
    for g in range(8):
        Wg[g * 16:(g + 1) * 16, g * 16:(g + 1) * 16] = 1.0
    c["Wg"] = _bf(Wg)
    # per-group sums landing at rows 32q+g for each PE row-tile offset q
    mask32 = np.zeros((128, 128), np.float32)
    for q in range(4):
        for g in range(8):
            mask32[g * 16:(g + 1) * 16, 32 * q + g] = 1.0
    c["mask32"] = _bf(mask32)
    c["magic"] = np.full((128, 1), 0x5f3759df, np.int32)
    c["b3c"] = _f32(np.tile(b3, 8)[:, None])

    # SE dense layers, per 128-channel slab; /4096 folds the HW mean.
    # f32 weights; rhs are 1-column f32 stat vectors.
    cg1 = np.zeros((128, 4, 32), np.float32)
    ga1 = np.zeros((128, 4, 32), np.float32)
    cg2 = np.zeros((32, 4, 128), np.float32)
    ga2 = np.zeros((32, 4, 128), np.float32)
    for s in range(4):
        cg1[:, s, :] = cg_w1[:, s * 128:(s + 1) * 128].T / 4096.0
        ga1[:, s, :] = ga_w1[:, s * 128:(s + 1) * 128].T / 4096.0
        cg2[:, s, :] = cg_w2[s * 128:(s + 1) * 128, :].T
        ga2[:, s, :] = ga_w2[s * 128:(s + 1) * 128, :].T
    c["cg1w"] = _bf(cg1.reshape(128, 128))
    c["ga1w"] = _bf(ga1.reshape(128, 128))
    c["cg2w"] = _bf(cg2.reshape(32, 512))
    c["ga2w"] = _bf(ga2.reshape(32, 512))
    c["cgb1"] = _f32(cg_b1[:, None])
    c["gab1"] = _f32(ga_b1[:, None])
    c["cgb2h"] = _f32(cg_b2.reshape(4, 128).T / 2.0)
    c["gab2h"] = _f32(ga_b2.reshape(4, 128).T / 2.0)

    c["gnw"] = _f32(np.tile(gn_w, 8)[:, None])
    c["gnb"] = _f32(np.tile(gn_b, 8)[:, None])
    return c


def build_program(gamma_f, mean_b3_f, const_shapes):
    nc = bass.Bass("TRN2", target_bir_lowering=False, debug=False,
                   num_devices=NCORES)
    xp_d = nc.dram_tensor("xp", [BPC, C, HP, WP], BF16, kind="ExternalInput")
    y_d = nc.dram_tensor("y", [BPC, C, H, W], BF16, kind="ExternalOutput")
    cd = {}
    for name, (shape, dt) in const_shapes.items():
        cd[name] = nc.dram_tensor(name, list(shape), dt, kind="ExternalInput")

    with TileContext(nc) as tc:
        with (
            tc.sbuf_pool(name="consts", bufs=1) as cpool,
            tc.sbuf_pool(name="big", bufs=2) as bpool,
            tc.sbuf_pool(name="fusedp", bufs=NSLAB + 1) as fpool,
            tc.sbuf_pool(name="small", bufs=2) as spool,
            tc.psum_pool(name="pwv", bufs=2) as pwv,
            tc.psum_pool(name="prep", bufs=1) as prep,
            tc.psum_pool(name="psmall", bufs=2) as psmall,
        ):
            cs = {}
            for name in const_shapes:
                t = cpool.tile(list(cd[name].shape), cd[name].dtype, name=f"c_{name}")
                nc.sync.dma_start(t[:, :], cd[name].ap())
                cs[name] = t

            emit_all(nc, tc, xp_d, y_d, cs, bpool, fpool, spool,
                     pwv, prep, psmall, gamma_f, mean_b3_f)
    _split_sync_waits(nc)
    return nc


def emit_all(nc, tc, xp_d, y_d, cs, bpool, fpool, spool,
             pwv, prep, psmall, gamma_f, mean_b3_f):
    """Software-pipelined emission: per-engine queues execute in program
    order, so stages of different (sample, slab) pairs are interleaved to
    fill each engine's stalls with ready work from other slabs."""
    sc = nc.scalar
    ve = nc.vector
    gp = nc.gpsimd
    te = nc.tensor

    def custom_ap(base_ap, extra_off, free_dims):
        p0 = list(base_ap.ap[0])
        return bass.AP(base_ap.tensor, base_ap.offset + extra_off,
                       [p0] + [list(d) for d in free_dims])

    # per-sample small stats tiles ([*, slab] columns)
    S = []
    for b in range(BPC):
        st = dict(
            xsum=spool.tile([128, 4], F32, name=f"xsum{b}"),
            x1sum=spool.tile([128, 4], F32, name=f"x1sum{b}"),
            x1sq=spool.tile([128, 4], F32, name=f"x1sq{b}"),
            fsum=spool.tile([128, 4], F32, name=f"fsum{b}"),
            edges=spool.tile([128, 16], BF16, name=f"edges{b}"),
            corners=spool.tile([128, 16], BF16, name=f"corners{b}"),
            mu=spool.tile([128, 4], F32, name=f"mu{b}"),
            rstdw=spool.tile([128, 4], F32, name=f"rstdw{b}"),
            b21=spool.tile([128, 4], F32, name=f"b21{b}"),
            biaswv=spool.tile([128, 4], F32, name=f"biaswv{b}"),
            scb=spool.tile([128, 4], F32, name=f"scb{b}"),
            gga=spool.tile([128, 4], F32, name=f"gga{b}"),
            fparts=spool.tile([128, 16], F32, name=f"fparts{b}"),
            xpad={}, strips={}, x1={}, fused={}, b21blk={}, sig={},
        )
        S.append(st)

    def stage_a(b, s):
        """Load slab, strip fold-trees, pooled sum, edge/corner gathers."""
        st = S[b]
        xpad = bpool.tile([128, HP * WP], BF16, name="xpad", tag="xpad",
                          bufs=5)
        st["xpad"][s] = xpad
        strips = spool.tile([128, 128], BF16, name="strips", tag="strips",
                            bufs=5)
        st["strips"][s] = strips
        srcp = xp_d.ap()[b, s * 128:(s + 1) * 128, :, :]
        srcp = bass.AP(srcp.tensor, srcp.offset,
                       [list(srcp.ap[0]), [1, HP * WP]])
        gp.dma_start(xpad[:, :], srcp)

        xp3 = xpad[:, :].rearrange("p (h w) -> p h w", w=WP)
        tree = spool.tile([128, 4352], BF16, name="tree", tag="tree",
                          bufs=2)
        # --- row strip (sum over w, h kept): fold w 64->32->16->8->4->2->1
        # interior rows are xp3[:, 1:65, 2:66]
        r1 = tree[:, 0:2048].rearrange("p (h w) -> p h w", w=32)
        ve.tensor_tensor(r1, xp3[:, 1:65, 2:34], xp3[:, 1:65, 34:66], OP.add)
        r2 = tree[:, 2048:3072].rearrange("p (h w) -> p h w", w=16)
        ve.tensor_tensor(r2, r1[:, :, 0:16], r1[:, :, 16:32], OP.add)
        r3 = tree[:, 3072:3584].rearrange("p (h w) -> p h w", w=8)
        ve.tensor_tensor(r3, r2[:, :, 0:8], r2[:, :, 8:16], OP.add)
        r4 = tree[:, 3584:3840].rearrange("p (h w) -> p h w", w=4)
        ve.tensor_tensor(r4, r3[:, :, 0:4], r3[:, :, 4:8], OP.add)
        r5 = tree[:, 3840:3968].rearrange("p (h w) -> p h w", w=2)
        ve.tensor_tensor(r5, r4[:, :, 0:2], r4[:, :, 2:4], OP.add)
        ve.tensor_tensor(strips[:, 0:64], r5[:, :, 0:1].rearrange(
            "p h w -> p (h w)"), r5[:, :, 1:2].rearrange("p h w -> p (h w)"),
            OP.add)
        # xsum (pooled sum of the slab) from the row strip
        ve.tensor_reduce(st["xsum"][:, s:s + 1], strips[:, 0:64], AX.X,
                         OP.add)
        # --- col strip (sum over h, w kept): fold h over the contiguous
        # padded buffer; pad columns ride along as zeros.
        # interior rows 1..64 inclusive -> fold rows (1..32)+(33..64)
        c1 = tree[:, 0:2176]
        ve.tensor_tensor(c1, xpad[:, WP:WP + 2176],
                         xpad[:, 33 * WP:33 * WP + 2176], OP.add)
        c2 = tree[:, 2176:3264]
        ve.tensor_tensor(c2, c1[:, 0:1088], c1[:, 1088:2176], OP.add)
        c3 = tree[:, 3264:3808]
        ve.tensor_tensor(c3, c2[:, 0:544], c2[:, 544:1088], OP.add)
        c4 = tree[:, 3808:4080]
        ve.tensor_tensor(c4, c3[:, 0:272], c3[:, 272:544], OP.add)
        c5 = tree[:, 4080:4216]
        ve.tensor_tensor(c5, c4[:, 0:136], c4[:, 136:272], OP.add)
        ve.tensor_tensor(strips[:, 64:128], c5[:, 2:66], c5[:, 70:134],
                         OP.add)
        # edge strip values {h=0, h=63, w=0, w=63}
        eap = custom_ap(strips[:, :], 0, [[64, 2], [63, 2]])
        ve.tensor_copy(
            st["edges"][:, 4 * s:4 * s + 4].rearrange("p (a c) -> p a c",
                                                      c=2), eap)
        # corner values {(0,0), (0,63), (63,0), (63,63)} (interior origin
        # at padded [1, 2])
        cap = custom_ap(xpad[:, :], WP + 2, [[63 * WP, 2], [63, 2]])
        ve.tensor_copy(
            st["corners"][:, 4 * s:4 * s + 4].rearrange("p (a c) -> p a c",
                                                        c=2), cap)

    def stage_se(b):
        """Channel SE gate (needs all 4 slab xsums)."""
        st = S[b]
        xsumbf = spool.tile([128, 4], BF16, name="xsumbf", tag="xsumbf")
        sc.activation(xsumbf[:, :], st["xsum"][:, :], AF.Copy)
        h1 = psmall.tile([32, 1], F32, name="h1", tag="ps_a")
        for s in range(NSLAB):
            te.matmul(h1[:, :], cs["cg1w"][:, 32 * s:32 * s + 32],
                      xsumbf[:, s:s + 1], start=(s == 0), stop=(s == 3))
        hid = spool.tile([32, 1], BF16, name="hid", tag="hid")
        sc.activation(hid[:, :], h1[:, :], AF.Relu, bias=cs["cgb1"][:, 0:1])
        gps = psmall.tile([128, 4], F32, name="gps", tag="ps_b")
        for s in range(NSLAB):
            te.matmul(gps[:, s:s + 1], cs["cg2w"][:, 128 * s:128 * s + 128],
                      hid[:, :], start=True, stop=True)
        for s in range(NSLAB):
            sc.activation(st["scb"][:, s:s + 1], gps[:, s:s + 1], AF.Tanh,
                          scale=0.5, bias=cs["cgb2h"][:, s:s + 1])
        # scb = gamma*sigmoid + 0.5 (the +0.5 completes the wv tanh->sigmoid
        # conversion inside the fused op; maskT4 carries the 0.5 factor)
        ve.tensor_scalar(st["scb"][:, :], st["scb"][:, :],
                         float(gamma_f) / 2.0, float(gamma_f) / 2.0 + 0.5,
                         OP.mult, OP.add)

    def stage_b(b, s):
        """Strip conv -> gates -> x1 map + its moment accumulators."""
        st = S[b]
        xpad = st["xpad"][s]
        strips = st["strips"][s]
        xp3 = xpad[:, :].rearrange("p (h w) -> p h w", w=WP)
        xpi3 = xp3[:, 1:H + 1, 2:W + 2]

        strip_ps = psmall.tile([128, 128], F32, name="strip_ps", tag="ps_a")
        te.matmul(strip_ps[:, :], cs["Wstrip"][:, :], strips[:, :],
                  start=True, stop=True)
        sgate = spool.tile([128, 128], BF16, name="sgate", tag="sgate",
                           bufs=3)
        sc.activation(sgate[:, :], strip_ps[:, :], AF.Tanh, scale=0.5,
                      bias=cs["b1t2"][:, 0:1])
        ve.tensor_scalar(sgate[:, :], sgate[:, :], 0.5, 0.5, OP.mult,
                         OP.add)

        x1 = bpool.tile([128, HW], BF16, name="x1", tag="x1", bufs=3)
        st["x1"][s] = x1
        x13 = x1[:, :].rearrange("p (h w) -> p h w", w=W)
        wg = sgate[:, 64:128].unsqueeze(1).broadcast_to((128, H, W))
        ve.tensor_tensor(x13, xpi3, wg, OP.mult)
        hg = sgate[:, 0:64].unsqueeze(2).broadcast_to((128, H, W))
        ve.scalar_tensor_tensor(x13, x13, 0.0, hg, OP.add, OP.mult,
                                accum_out=st["x1sum"][:, s:s + 1])
        junk = bpool.tile([128, HW], BF16, name="junk", tag="scratch",
                          bufs=2)
        sc.activation(junk[:, :], x1[:, :], AF.Square,
                      accum_out=st["x1sq"][:, s:s + 1])

    def stage_c(b, s):
        """GroupNorm stats, pooled-softmax coefficients, wv bias."""
        st = S[b]
        mu, rstdw, b21 = st["mu"], st["rstdw"], st["b21"]
        gp.tensor_scalar_mul(mu[:, s:s + 1], st["x1sum"][:, s:s + 1],
                             1.0 / HW)
        var = spool.tile([128, 1], F32, name="var", tag="var")
        gp.tensor_tensor(var[:, :], mu[:, s:s + 1], mu[:, s:s + 1], OP.mult)
        ve.scalar_tensor_tensor(var[:, :], st["x1sq"][:, s:s + 1], 1.0 / HW,
                                var[:, :], OP.mult, OP.subtract)
        ve.tensor_scalar_add(var[:, :], var[:, :], float(EPS))
        # rstd = rsqrt(var) via bit trick + 1 Newton iteration (~1.7e-3 rel)
        ti = spool.tile([128, 1], mybir.dt.int32, name="ti", tag="ti")
        ve.tensor_scalar(ti[:, :], var[:, :].bitcast(mybir.dt.int32), 1,
                         None, OP.logical_shift_right)
        ve.tensor_tensor(ti[:, :], cs["magic"][:, 0:1], ti[:, :],
                         OP.subtract)
        ry = spool.tile([128, 1], F32, name="ry", tag="ry")
        rt = spool.tile([128, 1], F32, name="rt", tag="rt")
        ve.tensor_copy(ry[:, :], ti[:, :].bitcast(F32))
        gp.tensor_tensor(rt[:, :], ry[:, :], ry[:, :], OP.mult)
        gp.tensor_tensor(rt[:, :], rt[:, :], var[:, :], OP.mult)
        gp.tensor_scalar(rt[:, :], rt[:, :], -0.5, 1.5, OP.mult, OP.add)
        gp.tensor_tensor(ry[:, :], ry[:, :], rt[:, :], OP.mult)
        gp.tensor_tensor(rstdw[:, s:s + 1], ry[:, :], cs["gnw"][:, 0:1],
                         OP.mult)

        # pooled x2 sums via edge algebra: A[p, tap]
        A = spool.tile([128, 9], F32, name="A", tag="A")
        Ap = A[:, :]
        gp.tensor_copy(Ap, st["xsum"][:, s:s + 1].broadcast_to((128, 9)))
        out_r = custom_ap(Ap, 3, [[3, 2], [1, 3]])
        in_r = custom_ap(st["edges"][:, :], 4 * s + 1, [[-1, 2], [0, 3]])
        gp.tensor_tensor(out_r, out_r, in_r, OP.subtract)
        out_c1 = custom_ap(Ap, 0, [[3, 3]])
        in_c1 = custom_ap(st["edges"][:, :], 4 * s + 3, [[0, 3]])
        gp.tensor_tensor(out_c1, out_c1, in_c1, OP.subtract)
        out_c2 = custom_ap(Ap, 2, [[3, 3]])
        in_c2 = custom_ap(st["edges"][:, :], 4 * s + 2, [[0, 3]])
        gp.tensor_tensor(out_c2, out_c2, in_c2, OP.subtract)
        out_k1 = custom_ap(Ap, 3, [[2, 2]])
        in_k1 = custom_ap(st["corners"][:, :], 4 * s + 3, [[-1, 2]])
        gp.tensor_tensor(out_k1, out_k1, in_k1, OP.add)
        out_k2 = custom_ap(Ap, 6, [[2, 2]])
        in_k2 = custom_ap(st["corners"][:, :], 4 * s + 1, [[-1, 2]])
        gp.tensor_tensor(out_k2, out_k2, in_k2, OP.add)
        Abf = spool.tile([128, 9], BF16, name="Abf", tag="Abf")
        sc.activation(Abf[:, :], A[:, :], AF.Copy)
        p2 = psmall.tile([128, 1], F32, name="p2", tag="ps_a")
        for t in range(9):
            te.matmul(p2[:, :], cs["W3sb"][:, 128 * t:128 * t + 128],
                      Abf[:, t:t + 1], start=(t == 0), stop=(t == 8))
        # e4 = exp(p2/HW + b3) via the ACT LUT
        e4 = spool.tile([128, 1], F32, name="e4", tag="e4")
        sc.activation(e4[:, :], p2[:, :], AF.Exp, scale=1.0 / HW,
                      bias=cs["b3c"][:, 0:1])
        ebf = spool.tile([128, 1], BF16, name="ebf", tag="ebf")
        sc.activation(ebf[:, :], e4[:, :], AF.Copy)
        # softmax denominator per group, replicated to all 128 rows
        rs8 = psmall.tile([128, 1], F32, name="rs8", tag="ps_b")
        te.matmul(rs8[:, :], cs["Wg"][:, :], ebf[:, :], start=True,
                  stop=True)
        rec = spool.tile([128, 1], F32, name="rec", tag="rec")
        ve.reciprocal(rec[:, :], rs8[:, :])
        a21 = spool.tile([128, 1], F32, name="a21", tag="a21")
        ve.tensor_tensor(a21[:, :], e4[:, :], rec[:, :], OP.mult)
        ve.tensor_tensor(b21[:, s:s + 1], a21[:, :], rstdw[:, s:s + 1],
                         OP.mult)
        kv = spool.tile([128, 1], F32, name="kv", tag="kv")
        gp.tensor_tensor(kv[:, :], b21[:, s:s + 1], mu[:, s:s + 1], OP.mult)
        k2 = spool.tile([128, 1], BF16, name="k2", tag="k2")
        ve.scalar_tensor_tensor(k2[:, :], a21[:, :], cs["gnb"][:, 0:1],
                                kv[:, :], OP.mult, OP.subtract)
        kps = psmall.tile([128, 1], F32, name="kps", tag="ps_a")
        te.matmul(kps[:, :], cs["mask32"][:, :], k2[:, :], start=True,
                  stop=True)
        # pre-halved for the tanh(scale=0.5) sigmoid form
        sc.activation(st["biaswv"][:, s:s + 1], kps[:, :], AF.Copy,
                      scale=0.5, bias=float(mean_b3_f) / 2.0)
        b21blk = spool.tile([128, 8], BF16, name="b21blk", tag="b21blk",
                            bufs=3)
        st["b21blk"][s] = b21blk
        sc.activation(b21blk[:, :], cs["mask8b"][:, :], AF.Copy,
                      scale=b21[:, s:s + 1])

    def stage_conv(b, s, half):
        """9-tap conv for one half (4 chunks of 8 rows) -- xpad only."""
        st = S[b]
        xpad = st["xpad"][s]
        xp3 = xpad[:, :].rearrange("p (h w) -> p h w", w=WP)
        wv = pwv.tile([128, 512], F32, name="wv", tag="wv")
        st.setdefault("wv", {})[(s, half)] = wv
        for t in range(9):
            dy, dx = TAPS[t]
            for q in range(4):
                hc = 4 * half + q
                h0 = hc * 8
                rhs = xp3[:, 1 + h0 + dy:1 + h0 + 8 + dy, 2 + dx:2 + dx + W]
                te.matmul(wv[32 * q:32 * q + 8, :],
                          cs["w3bar"][:, 8 * t:8 * t + 8], rhs,
                          start=(t == 0), stop=False,
                          tile_position=(0, 32 * q))

    def stage_b21(b, s, half):
        """10th tap: b21-weighted x1 contraction closes the wv accum."""
        st = S[b]
        x1 = st["x1"][s]
        b21blk = st["b21blk"][s]
        wv = st["wv"][(s, half)]
        for q in range(4):
            hc = 4 * half + q
            te.matmul(wv[32 * q:32 * q + 8, :], b21blk[:, :],
                      x1[:, 512 * hc:512 * hc + 512],
                      start=False, stop=True,
                      tile_position=(0, 32 * q))

    def stage_sig(b, s, half):
        """Sigmoid of wv (tanh form, per-block bias)."""
        st = S[b]
        wv = st["wv"][(s, half)]
        sig = spool.tile([128, 512], BF16, name="sig", tag="sig", bufs=4)
        st["sig"][(s, half)] = sig
        sc.activation(sig[:, :], wv[:, :], AF.Tanh, scale=0.5,
                      bias=st["biaswv"][:, s:s + 1])

    def stage_d(b, s, half):
        """Group->channel broadcast (PE) + fused map (V, 1 stt per half)."""
        st = S[b]
        xpad = st["xpad"][s]
        xp3 = xpad[:, :].rearrange("p (h w) -> p h w", w=WP)
        sig = st["sig"][(s, half)]
        if half == 0:
            fused = fpool.tile([128, HW], BF16, name="fused", tag="fused")
            st["fused"][s] = fused
        fused = st["fused"][s]
        for r in range(2):
            repq = prep.tile([128, 1024], F32, name="repq", tag="repq")
            for j in range(2):
                q = 2 * r + j
                te.matmul(repq[:, 512 * j:512 * j + 512],
                          cs["maskT4"][32 * q:32 * q + 8, :],
                          sig[32 * q:32 * q + 8, :],
                          start=True, stop=True,
                          tile_position=(32 * q, 0))
            h0 = 32 * half + 16 * r
            ve.scalar_tensor_tensor(
                fused[:, 1024 * (2 * half + r):1024 * (2 * half + r) + 1024],
                repq[:, :], st["scb"][:, s:s + 1],
                xp3[:, 1 + h0:1 + h0 + 16, 2:2 + W],
                OP.add, OP.mult,
                accum_out=st["fparts"][:, 4 * s + 2 * half + r:
                                       4 * s + 2 * half + r + 1])

    def stage_fs(b, s):
        """Slab fused-sum from the four quarter accumulators."""
        st = S[b]
        ve.tensor_reduce(st["fsum"][:, s:s + 1],
                         st["fparts"][:, 4 * s:4 * s + 4], AX.X, OP.add)

    def stage_ga(b):
        """Global-attn SE over fused."""
        st = S[b]
        fsumbf = spool.tile([128, 4], BF16, name="fsumbf", tag="fsumbf")
        sc.activation(fsumbf[:, :], st["fsum"][:, :], AF.Copy)
        h2 = psmall.tile([32, 1], F32, name="h2", tag="ps_a")
        for s in range(NSLAB):
            te.matmul(h2[:, :], cs["ga1w"][:, 32 * s:32 * s + 32],
                      fsumbf[:, s:s + 1], start=(s == 0), stop=(s == 3))
        hid2 = spool.tile([32, 1], BF16, name="hid2", tag="hid")
        sc.activation(hid2[:, :], h2[:, :], AF.Relu, bias=cs["gab1"][:, 0:1])
        gps2 = psmall.tile([128, 4], F32, name="gps2", tag="ps_b")
        for s in range(NSLAB):
            te.matmul(gps2[:, s:s + 1], cs["ga2w"][:, 128 * s:128 * s + 128],
                      hid2[:, :], start=True, stop=True)
        for s in range(NSLAB):
            sc.activation(st["gga"][:, s:s + 1], gps2[:, s:s + 1],
                          AF.Tanh, scale=0.5, bias=cs["gab2h"][:, s:s + 1])
        ve.tensor_scalar(st["gga"][:, :], st["gga"][:, :], 0.5, 0.5,
                         OP.mult, OP.add)

    def stage_e(b, s):
        """Final gate (V tensor_scalar, bf16 4x) + store."""
        st = S[b]
        fused = st["fused"][s]
        ve.tensor_scalar(fused[:, :], fused[:, :],
                         st["gga"][:, s:s + 1], None, OP.mult)
        dst = y_d.ap()[b, s * 128:(s + 1) * 128, :, :]
        dst = bass.AP(dst.tensor, dst.offset, [list(dst.ap[0]), [1, HW]])
        nc.sync.dma_start(dst, fused[:, :])

    A, Bst, Cs = stage_a, stage_b, stage_c
    CV, B21, SG, D, FS, E = (stage_conv, stage_b21, stage_sig, stage_d,
                             stage_fs, stage_e)
    SE, GA = stage_se, stage_ga
    sched = [
        (A, 0, 0), (A, 0, 1),
        (CV, 0, 0, 0), (Bst, 0, 0), (A, 0, 2),
        (CV, 0, 0, 1), (Bst, 0, 1), (Cs, 0, 0), (A, 0, 3), (SE, 0),
        (B21, 0, 0, 0), (SG, 0, 0, 0),
        (CV, 0, 1, 0), (B21, 0, 0, 1), (SG, 0, 0, 1),
        (Bst, 0, 2), (Cs, 0, 1),
        (D, 0, 0, 0), (CV, 0, 1, 1), (B21, 0, 1, 0), (SG, 0, 1, 0),
        (D, 0, 0, 1), (FS, 0, 0),
        (Bst, 0, 3), (Cs, 0, 2),
        (CV, 0, 2, 0), (B21, 0, 1, 1), (SG, 0, 1, 1),
        (D, 0, 1, 0), (A, 1, 0),
        (CV, 0, 2, 1), (B21, 0, 2, 0), (SG, 0, 2, 0),
        (D, 0, 1, 1), (FS, 0, 1), (Cs, 0, 3), (A, 1, 1),
        (CV, 0, 3, 0), (B21, 0, 2, 1), (SG, 0, 2, 1),
        (D, 0, 2, 0), (Bst, 1, 0), (A, 1, 2),
        (CV, 0, 3, 1), (B21, 0, 3, 0), (SG, 0, 3, 0),
        (D, 0, 2, 1), (FS, 0, 2), (Bst, 1, 1), (Cs, 1, 0), (A, 1, 3),
        (CV, 1, 0, 0), (B21, 0, 3, 1), (SG, 0, 3, 1),
        (D, 0, 3, 0), (SE, 1), (Bst, 1, 2), (Cs, 1, 1),
        (CV, 1, 0, 1), (B21, 1, 0, 0), (SG, 1, 0, 0),
        (D, 0, 3, 1), (FS, 0, 3), (GA, 0),
        (CV, 1, 1, 0), (B21, 1, 0, 1), (SG, 1, 0, 1),
        (D, 1, 0, 0), (E, 0, 0), (Bst, 1, 3), (Cs, 1, 2),
        (CV, 1, 1, 1), (B21, 1, 1, 0), (SG, 1, 1, 0),
        (D, 1, 0, 1), (FS, 1, 0), (E, 0, 1),
        (CV, 1, 2, 0), (B21, 1, 1, 1), (SG, 1, 1, 1),
        (D, 1, 1, 0), (E, 0, 2), (Cs, 1, 3),
        (CV, 1, 2, 1), (B21, 1, 2, 0), (SG, 1, 2, 0),
        (D, 1, 1, 1), (FS, 1, 1), (E, 0, 3),
        (CV, 1, 3, 0), (B21, 1, 2, 1), (SG, 1, 2, 1),
        (D, 1, 2, 0),
        (CV, 1, 3, 1), (B21, 1, 3, 0), (SG, 1, 3, 0),
        (D, 1, 2, 1), (FS, 1, 2),
        (B21, 1, 3, 1), (SG, 1, 3, 1),
        (D, 1, 3, 0),
        (D, 1, 3, 1), (FS, 1, 3), (GA, 1),
        (E, 1, 0), (E, 1, 1), (E, 1, 2), (E, 1, 3),
    ]
    for entry in sched:
        fn = entry[0]
        fn(*entry[1:])


def _ensure_ntff_hook():
    """run_bass_kernel_spmd(trace=True) under axon needs
    antenv.axon_hooks, which this image's antenv lacks. Shim it and
    register the ctypes-based NTFF hook from the boot package."""
    import types
    try:
        from antenv import axon_hooks  # noqa: F401
        return
    except ImportError:
        pass
    try:
        import antenv
        from trn_agent_boot.trn_boot import _ntff_profile_via_ctypes
        hooks = types.ModuleType("antenv.axon_hooks")
        _h = [None]
        hooks.set_axon_ntff_profile_hook = lambda h: _h.__setitem__(0, h)
        hooks.get_axon_ntff_profile_hook = lambda: _h[0]
        sys.modules["antenv.axon_hooks"] = hooks
        antenv.axon_hooks = hooks
        hooks.set_axon_ntff_profile_hook(
            _ntff_profile_via_ctypes("/opt/axon/libaxon_pjrt.so"))
    except Exception as e:  # profiling is best-effort
        print(f"ntff hook setup failed: {e}")


_CACHE = {}


def _get_program(consts, gamma_f, mean_b3_f):
    key = (float(gamma_f), float(mean_b3_f),
           tuple(sorted((k, v.tobytes()[:64].hex() if v.size > 16 else
                         v.tobytes().hex()) for k, v in consts.items())))
    key = hash(key)
    if key not in _CACHE:
        def _dt(v):
            if v.dtype == ml_dtypes.bfloat16:
                return BF16
            if v.dtype == np.int32:
                return mybir.dt.int32
            return F32
        const_shapes = {k: (v.shape, _dt(v)) for k, v in consts.items()}
        _CACHE[key] = build_program(gamma_f, mean_b3_f, const_shapes)
    return _CACHE[key]


def kernel(x, w1, b1, w3, b3, gn_w, gn_b, cg_w1, cg_b1, cg_w2, cg_b2,
           ga_w1, ga_b1, ga_w2, ga_b2, gamma, _return_timing=None):
    args = [np.asarray(a) for a in
            (x, w1, b1, w3, b3, gn_w, gn_b, cg_w1, cg_b1, cg_w2, cg_b2,
             ga_w1, ga_b1, ga_w2, ga_b2, gamma)]
    (x, w1, b1, w3, b3, gn_w, gn_b, cg_w1, cg_b1, cg_w2, cg_b2,
     ga_w1, ga_b1, ga_w2, ga_b2, gamma) = args
    consts = build_consts(w1, b1, w3, b3, gn_w, gn_b, cg_w1, cg_b1, cg_w2,
                          cg_b2, ga_w1, ga_b1, ga_w2, ga_b2, gamma)
    gamma_f = float(np.asarray(gamma).reshape(-1)[0])
    mean_b3_f = float(np.mean(b3))
    nc = _get_program(consts, gamma_f, mean_b3_f)

    xbf = x.astype(ml_dtypes.bfloat16)           # [B, C, H, W]
    xpad = np.zeros((B, C, HP, WP), ml_dtypes.bfloat16)
    xpad[:, :, 1:1 + H, 2:2 + W] = xbf

    in_maps = []
    for core in range(NCORES):
        sl = slice(core * BPC, (core + 1) * BPC)
        m = {"xp": np.ascontiguousarray(xpad[sl])}
        m.update(consts)
        in_maps.append(m)
    trace = bool(_return_timing is not None)
    if trace:
        _ensure_ntff_hook()
    last_err = None
    for _attempt in range(3):
        try:
            res = run_bass_kernel_spmd(nc, in_maps,
                                       core_ids=list(range(NCORES)),
                                       trace=trace)
            break
        except Exception as e:  # transient NRT device errors: retry
            last_err = e
    else:
        raise last_err
    if _return_timing is not None:
        _return_timing.update(dict(
            exec_time_ns=res.exec_time_ns,
            mean_exec_time_ns=res.mean_exec_time_ns,
        ))
    out = np.empty((B, C, H, W), np.float32)
    for core in range(NCORES):
        out[core * BPC:(core + 1) * BPC] = res.results[core]["y"].astype(
            np.float32)
    return out


# revision 15
# speedup vs baseline: 1.0538x; 1.0042x over previous
"""Trainium2 Bass kernel for nn_EMAX_60756607369740.

Computation (per sample b, per group g of 16 channels, over 64x64 maps):
  - coordinate pooling strips -> 1x1 conv (w1) -> sigmoid gates -> x1
  - per-channel GroupNorm stats of x1 (used only through the a21-weighted
    channel contraction; a11 == uniform 1/16 exactly because the GN output
    has zero spatial mean)
  - 3x3 conv branch x2 enters only via (1/16)*sum_c x2 and via its pooled
    per-channel sums (reconstructed algebraically from row/col/corner sums)
  - wv = (1/16)sum_c x2 + sum_c b21[c] x1[c] - k ; spatial = x*sigmoid(wv)
  - channel SE on x, fuse, global SE on fused.

Sharding: pure data parallel over batch B=16 -> 2 samples per core x 8 cores.
Per-core tile: [128 partitions = 8 groups x 16 ch, 4096 = 64h x 64w].

v2 layout/engine choices (vs the 233us baseline):
  - input staged host-side pre-padded [C, 66, 68] bf16 (1 h-pad row and
    2 w-pad cols per side) so every conv-tap matmul is a uniform in-bounds
    512-col rhs slice -- no edge clamping, no 448-col stragglers.
  - strip sums via deep bf16 fold trees at DVE 2x rate (no 1x
    tensor_reduce tails); strips kept bf16 so the strip-conv rhs needs no
    f32->bf16 staging copy.
  - repq (group->channel sigmoid broadcast) matmuls land in one 4-bank
    [128,2048] PSUM tile, so the fused-map scalar_tensor_tensor runs as 2
    ops/slab (PSUM 120-cyc startup paid 2x not 8x).
  - final gga gate on vector tensor_scalar (bf16 4x mode) for all slabs;
    scalar engine keeps Square/sigmoids only.
  - tiny stage-c chain ops placed per measured per-op cost (a21 multiply
    on DVE, not gpsimd).
  - all sigmoids tanh-based so the scalar engine stays in one ACT
    table-set (exp_and_others).
"""

import sys

for _p in ("/opt/trn_rl_repo", "/root/.axon_site/_ro/trn_rl_repo"):
    if _p not in sys.path:
        sys.path.insert(0, _p)

import numpy as np
import ml_dtypes

import bass_rust
import concourse.bass as bass
import concourse.mybir as mybir
from concourse.tile import TileContext
from concourse.bass_utils import run_bass_kernel_spmd

F32 = mybir.dt.float32
BF16 = mybir.dt.bfloat16
AF = mybir.ActivationFunctionType
OP = mybir.AluOpType
AX = mybir.AxisListType

B, C, H, W = 16, 512, 64, 64
G, CG, R = 32, 16, 16
EPS = 1e-5
NCORES = 8
BPC = B // NCORES          # samples per core
NSLAB = C // 128           # 4 slabs of 128 channels per sample
HW = H * W                 # 4096
WP = W + 4                 # padded row length 68 (2 left, 2 right zeros)
HP = H + 2                 # padded row count 66 (1 top, 1 bottom zero row)
# taps ordered dy=0 first so the first matmul in each PSUM group covers the
# full chunk (start=True clears the whole region)
TAPS = [(0, -1), (0, 0), (0, 1),
        (-1, -1), (-1, 0), (-1, 1),
        (1, -1), (1, 0), (1, 1)]

MAX_WAITS_PER_INST = 1


def _patched_drain_and_barrier(self, tick_clock, wait_clock):
    # Workaround for walrus "Too many sync wait commands" on the final tile
    # drain: split the aggregated sem waits across many drain instructions.
    drain_inst = self.nc.sync.drain()
    wait_clock.add_sem_waits(
        drain_inst.ins, bass_rust.ScopedClock({None: tick_clock.global_clock})
    )
    mi = drain_inst.ins
    si = mi.sync_info
    if si is not None and len(si.on_wait) > MAX_WAITS_PER_INST:
        waits = list(si.on_wait)
        mi.sync_info = bass_rust.SyncInfo(
            on_wait=waits[:MAX_WAITS_PER_INST], on_update=list(si.on_update)
        )
        rest = waits[MAX_WAITS_PER_INST:]
        for i in range(0, len(rest), MAX_WAITS_PER_INST):
            d2 = self.nc.sync.drain()
            d2.ins.sync_info = bass_rust.SyncInfo(
                on_wait=rest[i : i + MAX_WAITS_PER_INST], on_update=[]
            )
    self.nc.all_engine_barrier()
    popped = self.nc._tile_sem_poison_stack.pop()
    assert popped is self._sem_poison
    self.nc.clear_and_free_semaphores(list(self.sems.allocated().values()))
    self.nc.all_engine_barrier()


TileContext._drain_and_barrier = _patched_drain_and_barrier


def _split_sync_waits(nc, maxw=MAX_WAITS_PER_INST):
    """Walrus rejects instructions carrying more than a couple of sync
    waits. Rebuild each basic block, hoisting excess waits onto freshly
    created same-engine nops placed immediately before the instruction."""
    func = nc.m.functions[0]
    for blk in func.blocks:
        insts = list(blk.instructions)
        need = []
        for inst in insts:
            si = inst.sync_info
            if si is not None and len(si.on_wait) > maxw:
                need.append(inst)
        if not need:
            continue
        donors = {}
        for inst in need:
            si = inst.sync_info
            waits = list(si.on_wait)
            extra = waits[:-maxw] if maxw > 0 else waits
            keep = waits[-maxw:] if maxw > 0 else []
            inst.sync_info = bass_rust.SyncInfo(
                on_wait=keep, on_update=list(si.on_update))
            chunks = [extra[i:i + max(maxw, 1)]
                      for i in range(0, len(extra), max(maxw, 1))]
            nops = []
            for ch in chunks:
                bi = nc.engines[inst.engine].nop()
                ni = bi.ins
                ni.sync_info = bass_rust.SyncInfo(on_wait=ch, on_update=[])
                nops.append(ni)
                # the nop was appended to the current bb; pull it back out
                for fb in func.blocks:
                    fl = list(fb.instructions)
                    if fl and fl[-1] is ni:
                        fb.instructions = fl[:-1]
                        break
            donors[id(inst)] = nops
        out = []
        for inst in insts:
            out.extend(donors.get(id(inst), []))
            out.append(inst)
        blk.instructions = out


def _bf(x):
    return np.ascontiguousarray(x.astype(ml_dtypes.bfloat16))


def _f32(x):
    return np.ascontiguousarray(x.astype(np.float32))


def build_consts(w1, b1, w3, b3, gn_w, gn_b, cg_w1, cg_b1, cg_w2, cg_b2,
                 ga_w1, ga_b1, ga_w2, ga_b2, gamma):
    """Host-side weight transforms. All arrays laid out [partition, free]."""
    c = {}
    # strip 1x1 conv, block-diagonal over 8 groups; /64 folds the W (or H)
    # mean
    Wstrip = np.zeros((128, 128), np.float32)
    for g in range(8):
        # out[(g,o)] = sum_c w1[o,c] * strip[(g,c)] / 64
        Wstrip[g * 16:(g + 1) * 16, g * 16:(g + 1) * 16] = w1.T / 64.0
    c["Wstrip"] = _bf(Wstrip)
    # sigmoid(x) = 0.5*tanh(x/2) + 0.5 everywhere (keeps scalar in one ACT
    # table-set); biases pre-halved for the tanh(scale=0.5) form
    c["b1t2"] = _f32(np.tile(b1, 8)[:, None] / 2.0)

    # big conv tap weights for the pooled-sum reconstruction (1-col rhs)
    W3t = np.zeros((128, 9, 128), np.float32)
    for t, (dy, dx) in enumerate(TAPS):
        blk = w3[:, :, dy + 1, dx + 1].T  # [c_in, c_out]
        for g in range(8):
            W3t[g * 16:(g + 1) * 16, t, g * 16:(g + 1) * 16] = blk
    c["W3sb"] = _bf(W3t.reshape(128, 9 * 128))

    # channel-mean conv weights: out group column g, value sum_c w3 / 16.
    # 8-wide outputs keep each matmul inside one 32-col PE tile.
    w3bar = np.zeros((128, 9, 8), np.float32)
    for t, (dy, dx) in enumerate(TAPS):
        vec = w3[:, :, dy + 1, dx + 1].sum(0) / 16.0  # [c_in]
        for g in range(8):
            w3bar[g * 16:(g + 1) * 16, t, g] = vec
    c["w3bar"] = _bf(w3bar.reshape(128, 9 * 8))
    # maskT replicated at the 4 PE row-tile offsets so the 4 quarters'
    # broadcast matmuls run on (32,128) row tiles.  Entries are 0.5: the
    # broadcast turns tanh into sigmoid (0.5*T; the +0.5 rides in scb)
    maskT4 = np.zeros((128, 128), np.float32)
    for q in range(4):
        for g in range(8):
            maskT4[32 * q + g, g * 16:(g + 1) * 16] = 0.5
    c["maskT4"] = _bf(maskT4)
    mask8 = np.zeros((128, 8), np.float32)
    for g in range(8):
        mask8[g * 16:(g + 1) * 16, g] = 1.0
    c["mask8b"] = _bf(mask8)

    # group block mask: Wg[p, o] = 1 if same group (for per-group sums
    # replicated across the group's 16 channels)
    Wg = np.zeros((128, 128), np.float32)
    for g in range(8):
        Wg[g * 16:(g + 1) * 16, g * 16:(g + 1) * 16] = 1.0
    c["Wg"] = _bf(Wg)
    # per-group sums landing at rows 32q+g for each PE row-tile offset q
    mask32 = np.zeros((128, 128), np.float32)
    for q in range(4):
        for g in range(8):
            mask32[g * 16:(g + 1) * 16, 32 * q + g] = 1.0
    c["mask32"] = _bf(mask32)
    c["magic"] = np.full((128, 1), 0x5f3759df, np.int32)
    c["b3c"] = _f32(np.tile(b3, 8)[:, None])

    # SE dense layers, per 128-channel slab; /4096 folds the HW mean.
    # f32 weights; rhs are 1-column f32 stat vectors.
    cg1 = np.zeros((128, 4, 32), np.float32)
    ga1 = np.zeros((128, 4, 32), np.float32)
    cg2 = np.zeros((32, 4, 128), np.float32)
    ga2 = np.zeros((32, 4, 128), np.float32)
    for s in range(4):
        cg1[:, s, :] = cg_w1[:, s * 128:(s + 1) * 128].T / 4096.0
        ga1[:, s, :] = ga_w1[:, s * 128:(s + 1) * 128].T / 4096.0
        cg2[:, s, :] = cg_w2[s * 128:(s + 1) * 128, :].T
        ga2[:, s, :] = ga_w2[s * 128:(s + 1) * 128, :].T
    c["cg1w"] = _bf(cg1.reshape(128, 128))
    c["ga1w"] = _bf(ga1.reshape(128, 128))
    c["cg2w"] = _bf(cg2.reshape(32, 512))
    c["ga2w"] = _bf(ga2.reshape(32, 512))
    c["cgb1"] = _f32(cg_b1[:, None])
    c["gab1"] = _f32(ga_b1[:, None])
    c["cgb2h"] = _f32(cg_b2.reshape(4, 128).T / 2.0)
    c["gab2h"] = _f32(ga_b2.reshape(4, 128).T / 2.0)

    c["gnw"] = _f32(np.tile(gn_w, 8)[:, None])
    c["gnb"] = _f32(np.tile(gn_b, 8)[:, None])
    return c


# const packing: group small weight tensors into a few DMA-able blocks so
# the startup prologue is a handful of descriptors, not ~25 serial triggers
PACKS = (
    ("pk_bf128", BF16, ["Wstrip", "W3sb", "w3bar", "maskT4", "mask8b",
                        "Wg", "mask32", "cg1w", "ga1w"]),
    ("pk_f128", F32, ["b1t2", "b3c", "cgb2h", "gab2h", "gnw", "gnb"]),
    ("pk_bf32", BF16, ["cg2w", "ga2w"]),
    ("pk_f32s", F32, ["cgb1", "gab1"]),
)


class CView:
    """Column-offset view of a packed const tile, sliceable like a tile."""

    def __init__(self, tile, c0, w):
        self.tile, self.c0, self.w = tile, c0, w

    def __getitem__(self, idx):
        rs, cols = idx
        start = cols.start if cols.start is not None else 0
        stop = cols.stop if cols.stop is not None else self.w
        return self.tile[rs, self.c0 + start:self.c0 + stop]


def pack_consts(consts):
    """Host-side: concat const arrays into the PACKS blocks."""
    packed = {"magic": consts["magic"]}
    layout = {}
    for pname, _dt, names in PACKS:
        arrs = [consts[n] for n in names]
        off = 0
        for n, a in zip(names, arrs):
            layout[n] = (pname, off, a.shape[1])
            off += a.shape[1]
        packed[pname] = np.ascontiguousarray(np.concatenate(arrs, axis=1))
    return packed, layout


def build_program(gamma_f, mean_b3_f, packed_shapes, layout):
    nc = bass.Bass("TRN2", target_bir_lowering=False, debug=False,
                   num_devices=NCORES)
    xp_d = nc.dram_tensor("xp", [BPC, C, HP, WP], BF16, kind="ExternalInput")
    y_d = nc.dram_tensor("y", [BPC, C, H, W], BF16, kind="ExternalOutput")
    cd = {}
    for name, (shape, dt) in packed_shapes.items():
        cd[name] = nc.dram_tensor(name, list(shape), dt, kind="ExternalInput")

    with TileContext(nc) as tc:
        with (
            tc.sbuf_pool(name="consts", bufs=1) as cpool,
            tc.sbuf_pool(name="big", bufs=2) as bpool,
            tc.sbuf_pool(name="fusedp", bufs=NSLAB + 1) as fpool,
            tc.sbuf_pool(name="small", bufs=2) as spool,
            tc.psum_pool(name="pwv", bufs=2) as pwv,
            tc.psum_pool(name="prep", bufs=1) as prep,
            tc.psum_pool(name="psmall", bufs=2) as psmall,
        ):
            # prologue: kick off the first input slabs before the consts so
            # the vector engine's strip trees start ASAP
            pre = {}
            for (b0, s0) in ((0, 0), (0, 1)):
                xp_t = bpool.tile([128, HP * WP], BF16, name="xpad",
                                  tag="xpad", bufs=5)
                srcp = xp_d.ap()[b0, s0 * 128:(s0 + 1) * 128, :, :]
                srcp = bass.AP(srcp.tensor, srcp.offset,
                               [list(srcp.ap[0]), [1, HP * WP]])
                nc.sync.dma_start(xp_t[:, :], srcp)
                pre[(b0, s0)] = xp_t

            ptiles = {}
            for name in packed_shapes:
                t = cpool.tile(list(cd[name].shape), cd[name].dtype,
                               name=f"c_{name}")
                nc.sync.dma_start(t[:, :], cd[name].ap())
                ptiles[name] = t
            cs = {"magic": ptiles["magic"]}
            for n, (pname, off, w) in layout.items():
                cs[n] = CView(ptiles[pname], off, w)

            emit_all(nc, tc, xp_d, y_d, cs, bpool, fpool, spool,
                     pwv, prep, psmall, gamma_f, mean_b3_f, pre)
    _split_sync_waits(nc)
    return nc


def emit_all(nc, tc, xp_d, y_d, cs, bpool, fpool, spool,
             pwv, prep, psmall, gamma_f, mean_b3_f, pre):
    """Software-pipelined emission: per-engine queues execute in program
    order, so stages of different (sample, slab) pairs are interleaved to
    fill each engine's stalls with ready work from other slabs."""
    sc = nc.scalar
    ve = nc.vector
    gp = nc.gpsimd
    te = nc.tensor

    def custom_ap(base_ap, extra_off, free_dims):
        p0 = list(base_ap.ap[0])
        return bass.AP(base_ap.tensor, base_ap.offset + extra_off,
                       [p0] + [list(d) for d in free_dims])

    # per-sample small stats tiles ([*, slab] columns)
    S = []
    for b in range(BPC):
        st = dict(
            xsum=spool.tile([128, 4], F32, name=f"xsum{b}"),
            x1sum=spool.tile([128, 4], F32, name=f"x1sum{b}"),
            x1sq=spool.tile([128, 4], F32, name=f"x1sq{b}"),
            fsum=spool.tile([128, 4], F32, name=f"fsum{b}"),
            edges=spool.tile([128, 16], BF16, name=f"edges{b}"),
            corners=spool.tile([128, 16], BF16, name=f"corners{b}"),
            mu=spool.tile([128, 4], F32, name=f"mu{b}"),
            rstdw=spool.tile([128, 4], F32, name=f"rstdw{b}"),
            b21=spool.tile([128, 4], F32, name=f"b21{b}"),
            biaswv=spool.tile([128, 4], F32, name=f"biaswv{b}"),
            scb=spool.tile([128, 4], F32, name=f"scb{b}"),
            gga=spool.tile([128, 4], F32, name=f"gga{b}"),
            fparts=spool.tile([128, 16], F32, name=f"fparts{b}"),
            xpad={}, strips={}, x1={}, fused={}, b21blk={}, sig={},
        )
        S.append(st)

    def stage_a(b, s):
        """Load slab, strip fold-trees, pooled sum, edge/corner gathers."""
        st = S[b]
        if (b, s) in pre:
            xpad = pre.pop((b, s))
        else:
            xpad = bpool.tile([128, HP * WP], BF16, name="xpad", tag="xpad",
                              bufs=5)
            srcp = xp_d.ap()[b, s * 128:(s + 1) * 128, :, :]
            srcp = bass.AP(srcp.tensor, srcp.offset,
                           [list(srcp.ap[0]), [1, HP * WP]])
            gp.dma_start(xpad[:, :], srcp)
        st["xpad"][s] = xpad
        strips = spool.tile([128, 128], BF16, name="strips", tag="strips",
                            bufs=5)
        st["strips"][s] = strips

        xp3 = xpad[:, :].rearrange("p (h w) -> p h w", w=WP)
        tree = spool.tile([128, 4352], BF16, name="tree", tag="tree",
                          bufs=2)
        # --- row strip (sum over w, h kept): fold w 64->32->16->8->4->2->1
        # interior rows are xp3[:, 1:65, 2:66]
        r1 = tree[:, 0:2048].rearrange("p (h w) -> p h w", w=32)
        ve.tensor_tensor(r1, xp3[:, 1:65, 2:34], xp3[:, 1:65, 34:66], OP.add)
        r2 = tree[:, 2048:3072].rearrange("p (h w) -> p h w", w=16)
        ve.tensor_tensor(r2, r1[:, :, 0:16], r1[:, :, 16:32], OP.add)
        r3 = tree[:, 3072:3584].rearrange("p (h w) -> p h w", w=8)
        ve.tensor_tensor(r3, r2[:, :, 0:8], r2[:, :, 8:16], OP.add)
        r4 = tree[:, 3584:3840].rearrange("p (h w) -> p h w", w=4)
        ve.tensor_tensor(r4, r3[:, :, 0:4], r3[:, :, 4:8], OP.add)
        r5 = tree[:, 3840:3968].rearrange("p (h w) -> p h w", w=2)
        ve.tensor_tensor(r5, r4[:, :, 0:2], r4[:, :, 2:4], OP.add)
        ve.tensor_tensor(strips[:, 0:64], r5[:, :, 0:1].rearrange(
            "p h w -> p (h w)"), r5[:, :, 1:2].rearrange("p h w -> p (h w)"),
            OP.add)
        # xsum (pooled sum of the slab) from the row strip
        ve.tensor_reduce(st["xsum"][:, s:s + 1], strips[:, 0:64], AX.X,
                         OP.add)
        # --- col strip (sum over h, w kept): fold h over the contiguous
        # padded buffer; pad columns ride along as zeros.
        # interior rows 1..64 inclusive -> fold rows (1..32)+(33..64)
        c1 = tree[:, 0:2176]
        ve.tensor_tensor(c1, xpad[:, WP:WP + 2176],
                         xpad[:, 33 * WP:33 * WP + 2176], OP.add)
        c2 = tree[:, 2176:3264]
        ve.tensor_tensor(c2, c1[:, 0:1088], c1[:, 1088:2176], OP.add)
        c3 = tree[:, 3264:3808]
        ve.tensor_tensor(c3, c2[:, 0:544], c2[:, 544:1088], OP.add)
        c4 = tree[:, 3808:4080]
        ve.tensor_tensor(c4, c3[:, 0:272], c3[:, 272:544], OP.add)
        c5 = tree[:, 4080:4216]
        ve.tensor_tensor(c5, c4[:, 0:136], c4[:, 136:272], OP.add)
        ve.tensor_tensor(strips[:, 64:128], c5[:, 2:66], c5[:, 70:134],
                         OP.add)
        # edge strip values {h=0, h=63, w=0, w=63}
        eap = custom_ap(strips[:, :], 0, [[64, 2], [63, 2]])
        ve.tensor_copy(
            st["edges"][:, 4 * s:4 * s + 4].rearrange("p (a c) -> p a c",
                                                      c=2), eap)
        # corner values {(0,0), (0,63), (63,0), (63,63)} (interior origin
        # at padded [1, 2])
        cap = custom_ap(xpad[:, :], WP + 2, [[63 * WP, 2], [63, 2]])
        ve.tensor_copy(
            st["corners"][:, 4 * s:4 * s + 4].rearrange("p (a c) -> p a c",
                                                        c=2), cap)

    def stage_se(b):
        """Channel SE gate (needs all 4 slab xsums)."""
        st = S[b]
        xsumbf = spool.tile([128, 4], BF16, name="xsumbf", tag="xsumbf")
        sc.activation(xsumbf[:, :], st["xsum"][:, :], AF.Copy)
        h1 = psmall.tile([32, 1], F32, name="h1", tag="ps_a")
        for s in range(NSLAB):
            te.matmul(h1[:, :], cs["cg1w"][:, 32 * s:32 * s + 32],
                      xsumbf[:, s:s + 1], start=(s == 0), stop=(s == 3))
        hid = spool.tile([32, 1], BF16, name="hid", tag="hid")
        sc.activation(hid[:, :], h1[:, :], AF.Relu, bias=cs["cgb1"][:, 0:1])
        gps = psmall.tile([128, 4], F32, name="gps", tag="ps_b")
        for s in range(NSLAB):
            te.matmul(gps[:, s:s + 1], cs["cg2w"][:, 128 * s:128 * s + 128],
                      hid[:, :], start=True, stop=True)
        for s in range(NSLAB):
            sc.activation(st["scb"][:, s:s + 1], gps[:, s:s + 1], AF.Tanh,
                          scale=0.5, bias=cs["cgb2h"][:, s:s + 1])
        # scb = gamma*sigmoid + 0.5 (the +0.5 completes the wv tanh->sigmoid
        # conversion inside the fused op; maskT4 carries the 0.5 factor)
        ve.tensor_scalar(st["scb"][:, :], st["scb"][:, :],
                         float(gamma_f) / 2.0, float(gamma_f) / 2.0 + 0.5,
                         OP.mult, OP.add)

    def stage_b(b, s):
        """Strip conv -> gates -> x1 map + its moment accumulators."""
        st = S[b]
        xpad = st["xpad"][s]
        strips = st["strips"][s]
        xp3 = xpad[:, :].rearrange("p (h w) -> p h w", w=WP)
        xpi3 = xp3[:, 1:H + 1, 2:W + 2]

        strip_ps = psmall.tile([128, 128], F32, name="strip_ps", tag="ps_a")
        te.matmul(strip_ps[:, :], cs["Wstrip"][:, :], strips[:, :],
                  start=True, stop=True)
        sgate = spool.tile([128, 128], BF16, name="sgate", tag="sgate",
                           bufs=3)
        sc.activation(sgate[:, :], strip_ps[:, :], AF.Tanh, scale=0.5,
                      bias=cs["b1t2"][:, 0:1])
        ve.tensor_scalar(sgate[:, :], sgate[:, :], 0.5, 0.5, OP.mult,
                         OP.add)

        x1 = bpool.tile([128, HW], BF16, name="x1", tag="x1", bufs=3)
        st["x1"][s] = x1
        x13 = x1[:, :].rearrange("p (h w) -> p h w", w=W)
        wg = sgate[:, 64:128].unsqueeze(1).broadcast_to((128, H, W))
        ve.tensor_tensor(x13, xpi3, wg, OP.mult)
        hg = sgate[:, 0:64].unsqueeze(2).broadcast_to((128, H, W))
        ve.scalar_tensor_tensor(x13, x13, 0.0, hg, OP.add, OP.mult,
                                accum_out=st["x1sum"][:, s:s + 1])
        junk = bpool.tile([128, HW], BF16, name="junk", tag="scratch",
                          bufs=2)
        sc.activation(junk[:, :], x1[:, :], AF.Square,
                      accum_out=st["x1sq"][:, s:s + 1])

    def stage_c(b, s):
        """GroupNorm stats, pooled-softmax coefficients, wv bias."""
        st = S[b]
        mu, rstdw, b21 = st["mu"], st["rstdw"], st["b21"]
        gp.tensor_scalar_mul(mu[:, s:s + 1], st["x1sum"][:, s:s + 1],
                             1.0 / HW)
        var = spool.tile([128, 1], F32, name="var", tag="var")
        gp.tensor_tensor(var[:, :], mu[:, s:s + 1], mu[:, s:s + 1], OP.mult)
        ve.scalar_tensor_tensor(var[:, :], st["x1sq"][:, s:s + 1], 1.0 / HW,
                                var[:, :], OP.mult, OP.subtract)
        ve.tensor_scalar_add(var[:, :], var[:, :], float(EPS))
        # rstd = rsqrt(var) via bit trick + 1 Newton iteration (~1.7e-3 rel)
        ti = spool.tile([128, 1], mybir.dt.int32, name="ti", tag="ti")
        ve.tensor_scalar(ti[:, :], var[:, :].bitcast(mybir.dt.int32), 1,
                         None, OP.logical_shift_right)
        ve.tensor_tensor(ti[:, :], cs["magic"][:, 0:1], ti[:, :],
                         OP.subtract)
        ry = spool.tile([128, 1], F32, name="ry", tag="ry")
        rt = spool.tile([128, 1], F32, name="rt", tag="rt")
        ve.tensor_copy(ry[:, :], ti[:, :].bitcast(F32))
        gp.tensor_tensor(rt[:, :], ry[:, :], ry[:, :], OP.mult)
        gp.tensor_tensor(rt[:, :], rt[:, :], var[:, :], OP.mult)
        gp.tensor_scalar(rt[:, :], rt[:, :], -0.5, 1.5, OP.mult, OP.add)
        gp.tensor_tensor(ry[:, :], ry[:, :], rt[:, :], OP.mult)
        gp.tensor_tensor(rstdw[:, s:s + 1], ry[:, :], cs["gnw"][:, 0:1],
                         OP.mult)

        # pooled x2 sums via edge algebra: A[p, tap]
        A = spool.tile([128, 9], F32, name="A", tag="A")
        Ap = A[:, :]
        gp.tensor_copy(Ap, st["xsum"][:, s:s + 1].broadcast_to((128, 9)))
        out_r = custom_ap(Ap, 3, [[3, 2], [1, 3]])
        in_r = custom_ap(st["edges"][:, :], 4 * s + 1, [[-1, 2], [0, 3]])
        gp.tensor_tensor(out_r, out_r, in_r, OP.subtract)
        out_c1 = custom_ap(Ap, 0, [[3, 3]])
        in_c1 = custom_ap(st["edges"][:, :], 4 * s + 3, [[0, 3]])
        gp.tensor_tensor(out_c1, out_c1, in_c1, OP.subtract)
        out_c2 = custom_ap(Ap, 2, [[3, 3]])
        in_c2 = custom_ap(st["edges"][:, :], 4 * s + 2, [[0, 3]])
        gp.tensor_tensor(out_c2, out_c2, in_c2, OP.subtract)
        out_k1 = custom_ap(Ap, 3, [[2, 2]])
        in_k1 = custom_ap(st["corners"][:, :], 4 * s + 3, [[-1, 2]])
        gp.tensor_tensor(out_k1, out_k1, in_k1, OP.add)
        out_k2 = custom_ap(Ap, 6, [[2, 2]])
        in_k2 = custom_ap(st["corners"][:, :], 4 * s + 1, [[-1, 2]])
        gp.tensor_tensor(out_k2, out_k2, in_k2, OP.add)
        Abf = spool.tile([128, 9], BF16, name="Abf", tag="Abf")
        sc.activation(Abf[:, :], A[:, :], AF.Copy)
        p2 = psmall.tile([128, 1], F32, name="p2", tag="ps_a")
        for t in range(9):
            te.matmul(p2[:, :], cs["W3sb"][:, 128 * t:128 * t + 128],
                      Abf[:, t:t + 1], start=(t == 0), stop=(t == 8))
        # e4 = exp(p2/HW + b3) via the ACT LUT
        e4 = spool.tile([128, 1], F32, name="e4", tag="e4")
        sc.activation(e4[:, :], p2[:, :], AF.Exp, scale=1.0 / HW,
                      bias=cs["b3c"][:, 0:1])
        ebf = spool.tile([128, 1], BF16, name="ebf", tag="ebf")
        sc.activation(ebf[:, :], e4[:, :], AF.Copy)
        # softmax denominator per group, replicated to all 128 rows
        rs8 = psmall.tile([128, 1], F32, name="rs8", tag="ps_b")
        te.matmul(rs8[:, :], cs["Wg"][:, :], ebf[:, :], start=True,
                  stop=True)
        rec = spool.tile([128, 1], F32, name="rec", tag="rec")
        ve.reciprocal(rec[:, :], rs8[:, :])
        a21 = spool.tile([128, 1], F32, name="a21", tag="a21")
        ve.tensor_tensor(a21[:, :], e4[:, :], rec[:, :], OP.mult)
        ve.tensor_tensor(b21[:, s:s + 1], a21[:, :], rstdw[:, s:s + 1],
                         OP.mult)
        kv = spool.tile([128, 1], F32, name="kv", tag="kv")
        gp.tensor_tensor(kv[:, :], b21[:, s:s + 1], mu[:, s:s + 1], OP.mult)
        k2 = spool.tile([128, 1], BF16, name="k2", tag="k2")
        ve.scalar_tensor_tensor(k2[:, :], a21[:, :], cs["gnb"][:, 0:1],
                                kv[:, :], OP.mult, OP.subtract)
        kps = psmall.tile([128, 1], F32, name="kps", tag="ps_a")
        te.matmul(kps[:, :], cs["mask32"][:, :], k2[:, :], start=True,
                  stop=True)
        # pre-halved for the tanh(scale=0.5) sigmoid form
        sc.activation(st["biaswv"][:, s:s + 1], kps[:, :], AF.Copy,
                      scale=0.5, bias=float(mean_b3_f) / 2.0)
        b21blk = spool.tile([128, 8], BF16, name="b21blk", tag="b21blk",
                            bufs=3)
        st["b21blk"][s] = b21blk
        sc.activation(b21blk[:, :], cs["mask8b"][:, :], AF.Copy,
                      scale=b21[:, s:s + 1])

    def stage_conv(b, s, half):
        """9-tap conv for one half (4 chunks of 8 rows) -- xpad only."""
        st = S[b]
        xpad = st["xpad"][s]
        xp3 = xpad[:, :].rearrange("p (h w) -> p h w", w=WP)
        wv = pwv.tile([128, 512], F32, name="wv", tag="wv")
        st.setdefault("wv", {})[(s, half)] = wv
        for t in range(9):
            dy, dx = TAPS[t]
            for q in range(4):
                hc = 4 * half + q
                h0 = hc * 8
                rhs = xp3[:, 1 + h0 + dy:1 + h0 + 8 + dy, 2 + dx:2 + dx + W]
                te.matmul(wv[32 * q:32 * q + 8, :],
                          cs["w3bar"][:, 8 * t:8 * t + 8], rhs,
                          start=(t == 0), stop=False,
                          tile_position=(0, 32 * q))

    def stage_b21(b, s, half):
        """10th tap: b21-weighted x1 contraction closes the wv accum."""
        st = S[b]
        x1 = st["x1"][s]
        b21blk = st["b21blk"][s]
        wv = st["wv"][(s, half)]
        for q in range(4):
            hc = 4 * half + q
            te.matmul(wv[32 * q:32 * q + 8, :], b21blk[:, :],
                      x1[:, 512 * hc:512 * hc + 512],
                      start=False, stop=True,
                      tile_position=(0, 32 * q))

    def stage_sig(b, s, half):
        """Sigmoid of wv (tanh form, per-block bias)."""
        st = S[b]
        wv = st["wv"][(s, half)]
        sig = spool.tile([128, 512], BF16, name="sig", tag="sig", bufs=4)
        st["sig"][(s, half)] = sig
        sc.activation(sig[:, :], wv[:, :], AF.Tanh, scale=0.5,
                      bias=st["biaswv"][:, s:s + 1])

    def stage_d(b, s, half):
        """Group->channel broadcast (PE) + fused map (V, 1 stt per half)."""
        st = S[b]
        xpad = st["xpad"][s]
        xp3 = xpad[:, :].rearrange("p (h w) -> p h w", w=WP)
        sig = st["sig"][(s, half)]
        if half == 0:
            fused = fpool.tile([128, HW], BF16, name="fused", tag="fused")
            st["fused"][s] = fused
        fused = st["fused"][s]
        for r in range(2):
            repq = prep.tile([128, 1024], F32, name="repq", tag="repq")
            for j in range(2):
                q = 2 * r + j
                te.matmul(repq[:, 512 * j:512 * j + 512],
                          cs["maskT4"][32 * q:32 * q + 8, :],
                          sig[32 * q:32 * q + 8, :],
                          start=True, stop=True,
                          tile_position=(32 * q, 0))
            h0 = 32 * half + 16 * r
            ve.scalar_tensor_tensor(
                fused[:, 1024 * (2 * half + r):1024 * (2 * half + r) + 1024],
                repq[:, :], st["scb"][:, s:s + 1],
                xp3[:, 1 + h0:1 + h0 + 16, 2:2 + W],
                OP.add, OP.mult,
                accum_out=st["fparts"][:, 4 * s + 2 * half + r:
                                       4 * s + 2 * half + r + 1])

    def stage_fs(b, s):
        """Slab fused-sum from the four quarter accumulators."""
        st = S[b]
        ve.tensor_reduce(st["fsum"][:, s:s + 1],
                         st["fparts"][:, 4 * s:4 * s + 4], AX.X, OP.add)

    def stage_ga(b):
        """Global-attn SE over fused."""
        st = S[b]
        fsumbf = spool.tile([128, 4], BF16, name="fsumbf", tag="fsumbf")
        sc.activation(fsumbf[:, :], st["fsum"][:, :], AF.Copy)
        h2 = psmall.tile([32, 1], F32, name="h2", tag="ps_a")
        for s in range(NSLAB):
            te.matmul(h2[:, :], cs["ga1w"][:, 32 * s:32 * s + 32],
                      fsumbf[:, s:s + 1], start=(s == 0), stop=(s == 3))
        hid2 = spool.tile([32, 1], BF16, name="hid2", tag="hid")
        sc.activation(hid2[:, :], h2[:, :], AF.Relu, bias=cs["gab1"][:, 0:1])
        gps2 = psmall.tile([128, 4], F32, name="gps2", tag="ps_b")
        for s in range(NSLAB):
            te.matmul(gps2[:, s:s + 1], cs["ga2w"][:, 128 * s:128 * s + 128],
                      hid2[:, :], start=True, stop=True)
        for s in range(NSLAB):
            sc.activation(st["gga"][:, s:s + 1], gps2[:, s:s + 1],
                          AF.Tanh, scale=0.5, bias=cs["gab2h"][:, s:s + 1])
        ve.tensor_scalar(st["gga"][:, :], st["gga"][:, :], 0.5, 0.5,
                         OP.mult, OP.add)

    def stage_e(b, s):
        """Final gate (V tensor_scalar, bf16 4x) + store."""
        st = S[b]
        fused = st["fused"][s]
        ve.tensor_scalar(fused[:, :], fused[:, :],
                         st["gga"][:, s:s + 1], None, OP.mult)
        dst = y_d.ap()[b, s * 128:(s + 1) * 128, :, :]
        dst = bass.AP(dst.tensor, dst.offset, [list(dst.ap[0]), [1, HW]])
        nc.sync.dma_start(dst, fused[:, :])

    A, Bst, Cs = stage_a, stage_b, stage_c
    CV, B21, SG, D, FS, E = (stage_conv, stage_b21, stage_sig, stage_d,
                             stage_fs, stage_e)
    SE, GA = stage_se, stage_ga
    sched = [
        (A, 0, 0), (A, 0, 1),
        (CV, 0, 0, 0), (Bst, 0, 0), (A, 0, 2),
        (CV, 0, 0, 1), (Bst, 0, 1), (Cs, 0, 0), (A, 0, 3), (SE, 0),
        (B21, 0, 0, 0), (SG, 0, 0, 0),
        (CV, 0, 1, 0), (B21, 0, 0, 1), (SG, 0, 0, 1),
        (Bst, 0, 2), (Cs, 0, 1),
        (D, 0, 0, 0), (CV, 0, 1, 1), (B21, 0, 1, 0), (SG, 0, 1, 0),
        (D, 0, 0, 1), (FS, 0, 0),
        (Bst, 0, 3), (Cs, 0, 2),
        (CV, 0, 2, 0), (B21, 0, 1, 1), (SG, 0, 1, 1),
        (D, 0, 1, 0), (A, 1, 0),
        (CV, 0, 2, 1), (B21, 0, 2, 0), (SG, 0, 2, 0),
        (D, 0, 1, 1), (FS, 0, 1), (Cs, 0, 3), (A, 1, 1),
        (CV, 0, 3, 0), (B21, 0, 2, 1), (SG, 0, 2, 1),
        (D, 0, 2, 0), (Bst, 1, 0), (A, 1, 2),
        (CV, 0, 3, 1), (B21, 0, 3, 0), (SG, 0, 3, 0),
        (D, 0, 2, 1), (FS, 0, 2), (Bst, 1, 1), (Cs, 1, 0), (A, 1, 3),
        (CV, 1, 0, 0), (B21, 0, 3, 1), (SG, 0, 3, 1),
        (D, 0, 3, 0), (SE, 1), (Bst, 1, 2), (Cs, 1, 1),
        (CV, 1, 0, 1), (B21, 1, 0, 0), (SG, 1, 0, 0),
        (D, 0, 3, 1), (FS, 0, 3), (GA, 0),
        (CV, 1, 1, 0), (B21, 1, 0, 1), (SG, 1, 0, 1),
        (D, 1, 0, 0), (E, 0, 0), (Bst, 1, 3), (Cs, 1, 2),
        (CV, 1, 1, 1), (B21, 1, 1, 0), (SG, 1, 1, 0),
        (D, 1, 0, 1), (FS, 1, 0), (E, 0, 1),
        (CV, 1, 2, 0), (B21, 1, 1, 1), (SG, 1, 1, 1),
        (D, 1, 1, 0), (E, 0, 2), (Cs, 1, 3),
        (CV, 1, 2, 1), (B21, 1, 2, 0), (SG, 1, 2, 0),
        (D, 1, 1, 1), (FS, 1, 1), (E, 0, 3),
        (CV, 1, 3, 0), (B21, 1, 2, 1), (SG, 1, 2, 1),
        (D, 1, 2, 0),
        (CV, 1, 3, 1), (B21, 1, 3, 0), (SG, 1, 3, 0),
        (D, 1, 2, 1), (FS, 1, 2),
        (B21, 1, 3, 1), (SG, 1, 3, 1),
        (D, 1, 3, 0),
        (D, 1, 3, 1), (FS, 1, 3), (GA, 1),
        (E, 1, 0), (E, 1, 1), (E, 1, 2), (E, 1, 3),
    ]
    for entry in sched:
        fn = entry[0]
        fn(*entry[1:])


def _ensure_ntff_hook():
    """run_bass_kernel_spmd(trace=True) under axon needs
    antenv.axon_hooks, which this image's antenv lacks. Shim it and
    register the ctypes-based NTFF hook from the boot package."""
    import types
    try:
        from antenv import axon_hooks  # noqa: F401
        return
    except ImportError:
        pass
    try:
        import antenv
        from trn_agent_boot.trn_boot import _ntff_profile_via_ctypes
        hooks = types.ModuleType("antenv.axon_hooks")
        _h = [None]
        hooks.set_axon_ntff_profile_hook = lambda h: _h.__setitem__(0, h)
        hooks.get_axon_ntff_profile_hook = lambda: _h[0]
        sys.modules["antenv.axon_hooks"] = hooks
        antenv.axon_hooks = hooks
        hooks.set_axon_ntff_profile_hook(
            _ntff_profile_via_ctypes("/opt/axon/libaxon_pjrt.so"))
    except Exception as e:  # profiling is best-effort
        print(f"ntff hook setup failed: {e}")


_CACHE = {}


def _get_program(packed, layout, gamma_f, mean_b3_f):
    key = (float(gamma_f), float(mean_b3_f),
           tuple(sorted((k, v.tobytes()[:64].hex() if v.size > 16 else
                         v.tobytes().hex()) for k, v in packed.items())))
    key = hash(key)
    if key not in _CACHE:
        def _dt(v):
            if v.dtype == ml_dtypes.bfloat16:
                return BF16
            if v.dtype == np.int32:
                return mybir.dt.int32
            return F32
        packed_shapes = {k: (v.shape, _dt(v)) for k, v in packed.items()}
        _CACHE[key] = build_program(gamma_f, mean_b3_f, packed_shapes,
                                    layout)
    return _CACHE[key]


def kernel(x, w1, b1, w3, b3, gn_w, gn_b, cg_w1, cg_b1, cg_w2, cg_b2,
           ga_w1, ga_b1, ga_w2, ga_b2, gamma, _return_timing=None):
    args = [np.asarray(a) for a in
            (x, w1, b1, w3, b3, gn_w, gn_b, cg_w1, cg_b1, cg_w2, cg_b2,
             ga_w1, ga_b1, ga_w2, ga_b2, gamma)]
    (x, w1, b1, w3, b3, gn_w, gn_b, cg_w1, cg_b1, cg_w2, cg_b2,
     ga_w1, ga_b1, ga_w2, ga_b2, gamma) = args
    consts = build_consts(w1, b1, w3, b3, gn_w, gn_b, cg_w1, cg_b1, cg_w2,
                          cg_b2, ga_w1, ga_b1, ga_w2, ga_b2, gamma)
    packed, layout = pack_consts(consts)
    gamma_f = float(np.asarray(gamma).reshape(-1)[0])
    mean_b3_f = float(np.mean(b3))
    nc = _get_program(packed, layout, gamma_f, mean_b3_f)

    xbf = x.astype(ml_dtypes.bfloat16)           # [B, C, H, W]
    xpad = np.zeros((B, C, HP, WP), ml_dtypes.bfloat16)
    xpad[:, :, 1:1 + H, 2:2 + W] = xbf

    in_maps = []
    for core in range(NCORES):
        sl = slice(core * BPC, (core + 1) * BPC)
        m = {"xp": np.ascontiguousarray(xpad[sl])}
        m.update(packed)
        in_maps.append(m)
    trace = bool(_return_timing is not None)
    if trace:
        _ensure_ntff_hook()
    last_err = None
    for _attempt in range(3):
        try:
            res = run_bass_kernel_spmd(nc, in_maps,
                                       core_ids=list(range(NCORES)),
                                       trace=trace)
            break
        except Exception as e:  # transient NRT device errors: retry
            last_err = e
    else:
        raise last_err
    if _return_timing is not None:
        _return_timing.update(dict(
            exec_time_ns=res.exec_time_ns,
            mean_exec_time_ns=res.mean_exec_time_ns,
        ))
    out = np.empty((B, C, H, W), np.float32)
    for core in range(NCORES):
        out[core * BPC:(core + 1) * BPC] = res.results[core]["y"].astype(
            np.float32)
    return out
